# revision 1
# baseline (speedup 1.0000x reference)
"""2-layer GCN (GCNConv x2) on 8 Trainium2 NeuronCores.

Strategy (dst-sharded, edge-partitioned by destination):
- Each core owns N/8 destination nodes and the edges pointing at them.
- h~ = dinv * (x @ W1) computed per-shard, AllGathered to a full bf16 table.
- Per-edge messages fetched with dma_gather (4 SWDGE queues round-robin);
  scatter-add done as one-hot-indicator matmuls accumulating in PSUM
  (indicator = is_equal(iota, dstloc) * dinv[dst], built on DVE per chunk).
- Layer 1 accumulates transposed (aggT [hid, dst]) so bias+ReLU ride the
  activation engine per-partition and the block's h2 = out1 @ W2 matmul can
  consume it directly as lhsT; h2~ = dinv * h2 written f32, AllGathered,
  layer 2 repeats the same edge schedule against the h2 table.
"""
import sys
import types

import numpy as np
import ml_dtypes

P = 128
NCORES = 8
GMAX_CHUNKS = 32  # max chunks (128 idxs each) per dma_gather
SB_N = 6  # dst blocks per super-block (one PSUM bank each; 6+1+1 banks)
NQUEUES = 4

_CACHE = {}


# ---------------------------------------------------------------- compat ---
def _install_compat():
    """Patches for this axon/walrus stack (drain waits, per-inst wait caps,
    NTFF shim). Idempotent."""
    if _CACHE.get("compat"):
        return
    import concourse.tile as tile
    import concourse.mybir as mybir

    _ev = [0]

    def _split_inst_waits(ordered):
        for _bb, insts in ordered.items():
            out = []
            for inst in insts:
                si = getattr(inst, "sync_info", None)
                if si is not None and si.on_wait is not None and len(si.on_wait) > 1:
                    waits = list(si.on_wait)
                    excess, keep = waits[:-1], waits[-1:]
                    si.on_wait.clear()
                    for sw in keep:
                        si.on_wait.append(sw)
                    for i in range(0, len(excess), 2):
                        _ev[0] += 1
                        ev = mybir.InstEventSemaphore(
                            name=f"evsplit-{_ev[0]}", ins=[], outs=[]
                        )
                        ev.engine = inst.engine
                        ev.sync_info = mybir.SyncInfo(
                            on_wait=excess[i : i + 2], on_update=[]
                        )
                        out.append(ev)
                out.append(inst)
            insts[:] = out

    orig_lower = tile.TileContext._lower_ordered_insts

    def patched_lower(self, ordered):
        _split_inst_waits(ordered)
        return orig_lower(self, ordered)

    def patched_drain(self, tick_clock, wait_clock):
        sems_alloc = list(self.sems.allocated().values())
        carrier = self.nc.sync.wait_ge(sems_alloc[0], 0)
        wait_clock.add_sem_waits(
            carrier.ins, tile.ScopedClock({None: tick_clock.global_clock})
        )
        waits = list(carrier.ins.sync_info.on_wait)
        carrier.ins.sync_info.on_wait.clear()
        for sw in waits[:2]:
            carrier.ins.sync_info.on_wait.append(sw)
        for i in range(2, len(waits), 2):
            c = self.nc.sync.wait_ge(sems_alloc[0], 0)
            c.ins.sync_info.on_wait.clear()
            for sw in waits[i : i + 2]:
                c.ins.sync_info.on_wait.append(sw)
        self.nc.sync.drain(fusable=False)
        self.nc.all_engine_barrier()
        popped = self.nc._tile_sem_poison_stack.pop()
        assert popped is self._sem_poison
        self.nc.clear_and_free_semaphores(sems_alloc)
        self.nc.all_engine_barrier()

    tile.TileContext._lower_ordered_insts = patched_lower
    tile.TileContext._drain_and_barrier = patched_drain

    # NTFF profile hook shim (missing antenv.axon_hooks in this image)
    _hook = {}
    mod = types.ModuleType("antenv.axon_hooks")
    mod.set_axon_ntff_profile_hook = lambda h: _hook.update(hook=h)
    mod.get_axon_ntff_profile_hook = lambda: _hook.get("hook")
    sys.modules["antenv.axon_hooks"] = mod
    try:
        import antenv

        antenv.axon_hooks = mod
        from trn_agent_boot.trn_boot import _ntff_profile_via_ctypes

        mod.set_axon_ntff_profile_hook(
            _ntff_profile_via_ctypes("/opt/axon/libaxon_pjrt.so")
        )
    except Exception:
        pass
    _CACHE["compat"] = True


# ---------------------------------------------------------- preprocessing ---
class Schedule:
    pass


def _preprocess(n, edge_index):
    """Build the uniform cross-core schedule + per-core data streams."""
    shard = n // NCORES
    nblk = (shard + P - 1) // P
    nbanks = 4
    bank_rows = (n + nbanks - 1) // nbanks
    assert bank_rows <= 32767
    n_sb = (nblk + SB_N - 1) // SB_N

    src = edge_index[0].astype(np.int64)
    dst = edge_index[1].astype(np.int64)
    e = src.shape[0]
    deg = np.bincount(dst, minlength=n).astype(np.float64) + 1.0
    dinv = (1.0 / np.sqrt(deg)).astype(np.float32)

    # append self loops
    loops = np.arange(n, dtype=np.int64)
    src2 = np.concatenate([src, loops])
    dst2 = np.concatenate([dst, loops])

    core = dst2 // shard
    dl = dst2 - core * shard
    blk = dl // P
    dstloc = (dl % P).astype(np.int32)
    bank = src2 // bank_rows
    bidx = (src2 % bank_rows).astype(np.int32)

    # per-core counts per (block, bank)
    cnt = np.zeros((NCORES, nblk, nbanks), np.int64)
    flat = (core * nblk + blk) * nbanks + bank
    bc = np.bincount(flat, minlength=NCORES * nblk * nbanks)
    cnt[...] = bc.reshape(NCORES, nblk, nbanks)
    budget = np.ceil(cnt.max(axis=0) / P).astype(np.int64)  # [nblk, nbanks] chunks

    # schedule: for sb -> for bank -> for blk in sb (budget>0): chunks
    chunk_block = []  # global chunk idx -> block
    chunk_start = []
    chunk_stop = []
    gathers = []  # (col16_off, num_idxs, bank, chunk_off)
    seen_first = np.zeros(nblk, bool)
    # total chunks per block to detect last
    blk_total = budget.sum(axis=1)
    blk_done = np.zeros(nblk, np.int64)
    slot_off = 0
    sb_post = []  # per sb: list of blocks
    for s in range(n_sb):
        blocks = list(range(s * SB_N, min((s + 1) * SB_N, nblk)))
        for k in range(nbanks):
            seg = []  # (block, nchunks)
            for b in blocks:
                if budget[b, k] > 0:
                    seg.append((b, int(budget[b, k])))
            tot = sum(x[1] for x in seg)
            # split into gathers
            coff = len(chunk_block)
            for b, nch in seg:
                for j in range(nch):
                    chunk_block.append(b)
                    chunk_start.append(not seen_first[b])
                    seen_first[b] = True
                    blk_done[b] += 1
                    chunk_stop.append(blk_done[b] == blk_total[b])
            g0 = 0
            while g0 < tot:
                gn = min(GMAX_CHUNKS, tot - g0)
                gathers.append(
                    (slot_off // 16, gn * P, k, coff + g0)
                )
                slot_off += gn * P
                g0 += gn
        sb_post.append(blocks)

    totc = len(chunk_block)
    tot_slots = slot_off
    assert tot_slots == totc * P

    # per-core streams
    idx_stream = np.zeros((NCORES, 16, tot_slots // 16), np.int16)
    dstloc_s = np.full((NCORES, P, totc), -1.0, ml_dtypes.bfloat16)

    sb_arr = blk // SB_N
    order = np.lexsort((blk, bank, sb_arr, core))
    so_core = core[order]
    so_blk = blk[order]
    so_bank = bank[order]
    so_bidx = bidx[order]
    so_dstloc = dstloc[order]

    # walk schedule per core, consuming sorted runs
    ptr = np.searchsorted(so_core, np.arange(NCORES + 1))
    for c in range(NCORES):
        lo, hi = ptr[c], ptr[c + 1]
        cblk = so_blk[lo:hi]
        cbank = so_bank[lo:hi]
        cbidx = so_bidx[lo:hi]
        cdl = so_dstloc[lo:hi]
        csb = cblk // SB_N
        # group boundaries: runs of (sb, bank, blk) in this order already
        key = (csb * nbanks + cbank) * nblk + cblk
        # iterate schedule in same order
        pos = 0
        slot = 0
        idx_flat = np.zeros(tot_slots, np.int16)
        dl_flat = np.full(totc * P, -1.0, np.float32)
        for s in range(n_sb):
            blocks = list(range(s * SB_N, min((s + 1) * SB_N, nblk)))
            for k in range(nbanks):
                for b in blocks:
                    bud = int(budget[b, k])
                    if bud == 0:
                        continue
                    want = (s * nbanks + k) * nblk + b
                    cnt_cb = 0
                    while pos + cnt_cb < hi - lo and key[pos + cnt_cb] == want:
                        cnt_cb += 1
                    nsl = bud * P
                    idx_flat[slot : slot + cnt_cb] = cbidx[pos : pos + cnt_cb]
                    dl_flat[slot : slot + cnt_cb] = cdl[pos : pos + cnt_cb]
                    pos += cnt_cb
                    slot += nsl
        assert pos == hi - lo, (c, pos, hi - lo)
        assert slot == tot_slots
        # wrap: slot i -> idx[i%16, i//16] within each gather's window
        for (c16, nidx, _k, _coff) in gathers:
            sl = slice(c16 * 16, c16 * 16 + nidx)
            seg = idx_flat[sl].reshape(nidx // 16, 16).T  # [16, nidx/16]
            idx_stream[c][:, c16 : c16 + nidx // 16] = seg
        # dstloc layout: chunk C, partition p = slot C*128+p
        dstloc_s[c] = dl_flat.reshape(totc, P).T.astype(ml_dtypes.bfloat16)

    sch = Schedule()
    sch.n, sch.e, sch.shard, sch.nblk, sch.nbanks = n, e, shard, nblk, nbanks
    sch.bank_rows, sch.n_sb, sch.totc = bank_rows, n_sb, totc
    sch.tot_slots = tot_slots
    sch.chunk_block = chunk_block
    sch.chunk_start = chunk_start
    sch.chunk_stop = chunk_stop
    sch.gathers = gathers
    sch.sb_post = sb_post
    sch.budget = budget
    sch.dinv = dinv
    sch.idx_stream = np.tile(idx_stream, (1, 8, 1))  # replicate to 128 partitions
    sch.dstloc_s = dstloc_s
    return sch


# ----------------------------------------------------------------- build ---
def _build(sch, in_dim, hid, out_dim):
    import concourse.mybir as mybir
    import concourse.tile as tile
    from concourse import bacc

    bf16 = mybir.dt.bfloat16
    f32 = mybir.dt.float32
    shard, nblk, nbanks = sch.shard, sch.nblk, sch.nbanks
    totc, n_sb = sch.totc, sch.n_sb
    n = sch.n

    nc = bacc.Bacc(num_swdge_queues=NQUEUES)

    xT = nc.declare_dram_parameter("xT", [in_dim, shard], bf16, isOutput=False)
    idxs = nc.declare_dram_parameter(
        "idxs", [P, sch.tot_slots // 16], mybir.dt.int16, isOutput=False
    )
    dstloc = nc.declare_dram_parameter("dstloc", [P, totc], bf16, isOutput=False)
    iotar_in = nc.declare_dram_parameter("iotar", [P, GMAX_CHUNKS * P], bf16, isOutput=False)
    dinvbc = nc.declare_dram_parameter("dinvbc", [P, nblk * P], f32, isOutput=False)
    dinvb = nc.declare_dram_parameter("dinvb", [P, nblk], f32, isOutput=False)
    w1 = nc.declare_dram_parameter("W1", [in_dim, hid], bf16, isOutput=False)
    b1 = nc.declare_dram_parameter("b1", [hid, 1], f32, isOutput=False)
    w2 = nc.declare_dram_parameter("W2", [hid, out_dim], bf16, isOutput=False)
    b2bc = nc.declare_dram_parameter("b2bc", [P, out_dim], f32, isOutput=False)
    iota_in = nc.declare_dram_parameter("iota", [P, P], bf16, isOutput=False)
    out_ext = nc.declare_dram_parameter("out", [shard, out_dim], f32, isOutput=True)

    hloc = nc.dram_tensor("hloc", [shard, hid], bf16)
    hfull = nc.dram_tensor("hfull", [n, hid], bf16, addr_space="Shared")
    h2loc = nc.dram_tensor("h2loc", [shard, P], bf16)
    h2full = nc.dram_tensor("h2full", [n, P], bf16, addr_space="Shared")

    kin = in_dim // P  # contraction tiles for layer-1 matmul

    with tile.TileContext(nc) as tc:
        with (
            tc.tile_pool(name="const", bufs=1) as cpool,
            tc.tile_pool(name="xload", bufs=2) as xpool,
            tc.tile_pool(name="hb", bufs=2) as hbpool,
            tc.tile_pool(name="idx", bufs=4) as ipool,
            tc.tile_pool(name="gath", bufs=6) as gpool,
            tc.tile_pool(name="sind", bufs=4) as spool,
            tc.tile_pool(name="conv", bufs=8) as vpool,
            tc.tile_pool(name="blk", bufs=3) as bpool,
            tc.tile_pool(name="psh", bufs=1, space="PSUM") as psh,
            tc.tile_pool(name="psagg", bufs=6, space="PSUM") as psagg,
            tc.tile_pool(name="psh2", bufs=1, space="PSUM") as psh2,
        ):
            # one register per distinct gather size, set once
            import contextlib

            regstack = contextlib.ExitStack()
            nidx_vals = sorted({g[1] for g in sch.gathers})
            nreg_map = {}
            for v in nidx_vals:
                r = regstack.enter_context(nc.gpsimd.register(f"nreg_{v}"))
                nc.gpsimd.reg_mov(r, v)
                nreg_map[v] = r
            # ---- constants into SBUF
            iota_sb = cpool.tile([P, P], bf16, tag="iota")
            nc.sync.dma_start(out=iota_sb[:], in_=iota_in[:])
            w1_t = [cpool.tile([P, hid], bf16, tag=f"w1_{k}", name=f"w1t{k}") for k in range(kin)]
            for k in range(kin):
                nc.sync.dma_start(out=w1_t[k][:], in_=w1[k * P : (k + 1) * P, :])
            w2_sb = cpool.tile([hid, out_dim], bf16, tag="w2")
            nc.sync.dma_start(out=w2_sb[:], in_=w2[:])
            b1_sb = cpool.tile([hid, 1], f32, tag="b1")
            nc.sync.dma_start(out=b1_sb[:], in_=b1[:])
            b2_sb = cpool.tile([P, out_dim], f32, tag="b2")
            nc.sync.dma_start(out=b2_sb[:], in_=b2bc[:])
            dinvb_sb = cpool.tile([P, nblk], f32, tag="dinvb")
            nc.sync.dma_start(out=dinvb_sb[:], in_=dinvb[:])
            dstloc_sb = cpool.tile([P, totc], bf16, tag="dstloc")
            nc.sync.dma_start(out=dstloc_sb[:], in_=dstloc[:])
            iotar_sb = cpool.tile([P, GMAX_CHUNKS * P], bf16, tag="iotar")
            nc.sync.dma_start(out=iotar_sb[:], in_=iotar_in[:])

            # ---- h~ = dinv * (x @ W1), shard-local, bf16
            XGRP = 8  # blocks of columns per xT load
            for g0 in range(0, nblk, XGRP):
                g1 = min(g0 + XGRP, nblk)
                c0, c1 = g0 * P, min(g1 * P, shard)
                xt = [
                    xpool.tile([P, c1 - c0], bf16, tag=f"xt{k}", name=f"xt{k}")
                    for k in range(kin)
                ]
                for k in range(kin):
                    nc.sync.dma_start(
                        out=xt[k][:], in_=xT[k * P : (k + 1) * P, c0:c1]
                    )
                for b in range(g0, g1):
                    m = min(P, shard - b * P)
                    hp = psh.tile([P, hid], f32, tag="hps")
                    for k in range(kin):
                        nc.tensor.matmul(
                            out=hp[:m, :],
                            lhsT=xt[k][:, b * P - c0 : b * P - c0 + m],
                            rhs=w1_t[k][:],
                            start=(k == 0),
                            stop=(k == kin - 1),
                        )
                    hsb = hbpool.tile([P, hid], bf16, tag="hsb")
                    nc.scalar.activation(
                        out=hsb[:m, :],
                        in_=hp[:m, :],
                        func=mybir.ActivationFunctionType.Copy,
                        scale=dinvb_sb[:m, b : b + 1],
                    )
                    nc.sync.dma_start(
                        out=hloc[b * P : b * P + m, :], in_=hsb[:m, :]
                    )

            nc.gpsimd.collective_compute(
                "AllGather",
                mybir.AluOpType.bypass,
                ins=[hloc[:]],
                outs=[hfull[:]],
                replica_groups=[list(range(NCORES))],
            )

            # ---- layer pipelines
            def run_layer(layer):
                table = hfull if layer == 1 else h2full
                gq = [0]
                for s in range(n_sb):
                    blocks = sch.sb_post[s]
                    w = P if layer == 1 else out_dim
                    agg_t = {
                        b: psagg.tile([P, w], f32, tag="agg", name=f"agg{s}_{b}")
                        for b in blocks
                    }

                    def slot(b):
                        return agg_t[b][:, :]

                    blocks_set = set(blocks)
                    for (c16, nidx, k, coff) in [
                        g
                        for g in sch.gathers
                        if sch.chunk_block[g[3]] in blocks_set
                    ]:
                        nch = nidx // P
                        it = ipool.tile([P, GMAX_CHUNKS * 8], mybir.dt.int16, tag="it")
                        nc.sync.dma_start(
                            out=it[:, : nidx // 16],
                            in_=idxs[:, c16 : c16 + nidx // 16],
                        )
                        gt = gpool.tile([P, GMAX_CHUNKS, P], bf16, tag="gt")
                        r0 = k * sch.bank_rows
                        r1 = min(r0 + sch.bank_rows, n)
                        nc.gpsimd.dma_gather(
                            out_ap=gt[:, :nch, :],
                            in_ap=table[r0:r1, :],
                            idxs_ap=it[:, : nidx // 16],
                            num_idxs=nidx,
                            num_idxs_reg=nreg_map[nidx],
                            elem_size=P,
                            single_packet=False,
                            queue_num=gq[0] % NQUEUES,
                        )
                        gq[0] += 1
                        # one batched 0/1 indicator build per gather
                        sbig = spool.tile([P, GMAX_CHUNKS, P], bf16, tag="sind")
                        nc.vector.tensor_tensor(
                            out=sbig[:, :nch, :],
                            in0=iotar_sb[:, : nch * P].rearrange(
                                "p (k f) -> p k f", k=nch
                            ),
                            in1=dstloc_sb[:, coff : coff + nch].to_broadcast(
                                [P, nch, P]
                            ),
                            op=mybir.AluOpType.is_equal,
                        )
                        for j in range(nch):
                            C = coff + j
                            b = sch.chunk_block[C]
                            st = sch.chunk_start[C]
                            sp = sch.chunk_stop[C]
                            if layer == 1:
                                nc.tensor.matmul(
                                    out=slot(b),
                                    lhsT=gt[:, j, :],
                                    rhs=sbig[:, j, :],
                                    start=st,
                                    stop=sp,
                                )
                            else:
                                nc.tensor.matmul(
                                    out=slot(b),
                                    lhsT=sbig[:, j, :],
                                    rhs=gt[:, j, :out_dim],
                                    start=st,
                                    stop=sp,
                                )
                    # ---- block epilogue for this super-block
                    for b in blocks:
                        m = min(P, shard - b * P)
                        if layer == 1:
                            dv = bpool.tile([P, P], f32, tag="dv")
                            nc.sync.dma_start(
                                out=dv[:], in_=dinvbc[:, b * P : (b + 1) * P]
                            )
                            t1 = bpool.tile([P, P], bf16, tag="t1")
                            nc.vector.tensor_tensor(
                                out=t1[:],
                                in0=slot(b),
                                in1=dv[:],
                                op=mybir.AluOpType.mult,
                            )
                            o1 = bpool.tile([P, P], bf16, tag="o1")
                            nc.scalar.activation(
                                out=o1[:],
                                in_=t1[:],
                                func=mybir.ActivationFunctionType.Relu,
                                bias=b1_sb[:, :1],
                            )
                            h2p = psh2.tile([P, out_dim], f32, tag="h2p")
                            nc.tensor.matmul(
                                out=h2p[:],
                                lhsT=o1[:],
                                rhs=w2_sb[:],
                                start=True,
                                stop=True,
                            )
                            h2s = bpool.tile([P, P], bf16, tag="h2s")
                            nc.vector.memset(h2s[:, out_dim:], 0.0)
                            nc.scalar.activation(
                                out=h2s[:m, :out_dim],
                                in_=h2p[:m, :],
                                func=mybir.ActivationFunctionType.Copy,
                                scale=dinvb_sb[:m, b : b + 1],
                            )
                            nc.sync.dma_start(
                                out=h2loc[b * P : b * P + m, :], in_=h2s[:m, :]
                            )
                        else:
                            t2 = bpool.tile([P, out_dim], f32, tag="t2")
                            nc.scalar.activation(
                                out=t2[:m, :],
                                in_=slot(b)[:m, :],
                                func=mybir.ActivationFunctionType.Copy,
                                scale=dinvb_sb[:m, b : b + 1],
                            )
                            ob = bpool.tile([P, out_dim], f32, tag="ob")
                            nc.vector.tensor_tensor(
                                out=ob[:m, :],
                                in0=t2[:m, :],
                                in1=b2_sb[:m, :],
                                op=mybir.AluOpType.add,
                            )
                            nc.sync.dma_start(
                                out=out_ext[b * P : b * P + m, :], in_=ob[:m, :]
                            )

            run_layer(1)
            nc.gpsimd.collective_compute(
                "AllGather",
                mybir.AluOpType.bypass,
                ins=[h2loc[:]],
                outs=[h2full[:]],
                replica_groups=[list(range(NCORES))],
            )
            run_layer(2)
            regstack.close()

    nc.compile()
    return nc


# ---------------------------------------------------------------- kernel ---
def _make_in_maps(sch, x, W1, b1v, W2, b2v):
    hid = W1.shape[1]
    out_dim = W2.shape[1]
    shard = sch.shard
    bf = ml_dtypes.bfloat16
    in_maps = []
    w1b = W1.astype(bf)
    w2b = W2.astype(bf)
    b1c = b1v.reshape(hid, 1).astype(np.float32).copy()
    b2c = np.broadcast_to(b2v.astype(np.float32), (P, out_dim)).copy()
    iota = np.broadcast_to(np.arange(P, dtype=np.float32), (P, P)).astype(bf)
    iotar = np.tile(np.arange(P, dtype=np.float32), (P, GMAX_CHUNKS)).astype(bf)
    for c in range(NCORES):
        xs = np.ascontiguousarray(x[c * shard : (c + 1) * shard].astype(bf).T)
        dv = sch.dinv[c * shard : (c + 1) * shard]
        full = np.zeros(sch.nblk * P, np.float32)
        full[:shard] = dv
        dvb = np.ascontiguousarray(full.reshape(sch.nblk, P).T)
        dbc = np.broadcast_to(full, (P, sch.nblk * P)).copy()
        in_maps.append(
            {
                "xT": xs,
                "idxs": sch.idx_stream[c],
                "dstloc": sch.dstloc_s[c],
                "dinvb": dvb,
                "W1": w1b,
                "b1": b1c,
                "W2": w2b,
                "b2bc": b2c,
                "iota": iota,
                "iotar": iotar,
                "dinvbc": dbc,
            }
        )
    return in_maps


def _get_compiled(n, e, edge_index, in_dim, hid, out_dim):
    key = ("nc", n, e)
    if key not in _CACHE:
        sch = _preprocess(n, edge_index)
        _CACHE[("sched", n, e)] = sch
        _CACHE[key] = _build(sch, in_dim, hid, out_dim)
    return _CACHE[("sched", n, e)], _CACHE[key]


def kernel(x, edge_index, W1, b1, W2, b2):
    _install_compat()
    from concourse.bass_utils import run_bass_kernel_spmd

    x = np.asarray(x)
    edge_index = np.asarray(edge_index)
    W1 = np.asarray(W1, np.float32)
    b1v = np.asarray(b1, np.float32)
    W2 = np.asarray(W2, np.float32)
    b2v = np.asarray(b2, np.float32)
    n, in_dim = x.shape
    hid = W1.shape[1]
    out_dim = W2.shape[1]

    sch, nc = _get_compiled(n, edge_index.shape[1], edge_index, in_dim, hid, out_dim)
    in_maps = _make_in_maps(sch, x, W1, b1v, W2, b2v)
    import os

    trace = bool(os.environ.get("GCN_TRACE"))
    res = run_bass_kernel_spmd(
        nc, in_maps, core_ids=list(range(NCORES)), trace=trace
    )
    global LAST_EXEC_NS
    LAST_EXEC_NS = res.exec_time_ns
    return np.concatenate([res.results[c]["out"] for c in range(NCORES)], axis=0)


LAST_EXEC_NS = None



# revision 50
# speedup vs baseline: 2.4666x; 2.4666x over previous
"""2-layer GCN (GCNConv x2) on 8 Trainium2 NeuronCores.

Strategy (dst-sharded, edge-partitioned by destination):
- Each core owns N/8 destination nodes and the edges pointing at them.
- h~ = dinv * (x @ W1) computed per-shard, AllGathered to a full bf16 table.
- Per-edge messages fetched with dma_gather (4 SWDGE queues round-robin);
  scatter-add done as one-hot-indicator matmuls accumulating in PSUM.
- Q7 descriptor generation is the critical resource, so the schedule
  minimizes gathered slots: no self-loop edges (self term is an
  identity-matmul add of SBUF-resident rows), per-(superblock, bank)
  chunking with per-block max-core slot packing, trailing -1 indices so
  each core only generates descriptors for its real edge count.
- Epilogues run on Scalar/TensorE only (relu(dinv*x) = dinv*relu(x));
  DVE just builds indicator matrices (batched is_equal per gather).
"""
import sys
import types

import numpy as np
import ml_dtypes

P = 128
NCORES = 8
GMAX_CHUNKS = 40  # max chunks (128 idxs each) per dma_gather
SB_N = 7  # dst blocks per super-block = PSUM accumulation banks
NQUEUES = 4
IOTAR_K = 48  # max instances per batched is_eq

_CACHE = {}


# ---------------------------------------------------------------- compat ---
def _install_compat():
    """Patches for this axon/walrus stack (drain waits, per-inst wait caps,
    NTFF shim). Idempotent."""
    if _CACHE.get("compat"):
        return
    import concourse.tile as tile
    import concourse.mybir as mybir

    _ev = [0]

    def _split_inst_waits(ordered):
        for _bb, insts in ordered.items():
            out = []
            for inst in insts:
                si = getattr(inst, "sync_info", None)
                if si is not None and si.on_wait is not None and len(si.on_wait) > 1:
                    waits = list(si.on_wait)
                    excess, keep = waits[:-1], waits[-1:]
                    si.on_wait.clear()
                    for sw in keep:
                        si.on_wait.append(sw)
                    for i in range(0, len(excess), 2):
                        _ev[0] += 1
                        ev = mybir.InstEventSemaphore(
                            name=f"evsplit-{_ev[0]}", ins=[], outs=[]
                        )
                        ev.engine = inst.engine
                        ev.sync_info = mybir.SyncInfo(
                            on_wait=excess[i : i + 2], on_update=[]
                        )
                        out.append(ev)
                out.append(inst)
            insts[:] = out

    orig_lower = tile.TileContext._lower_ordered_insts

    def patched_lower(self, ordered):
        _split_inst_waits(ordered)
        return orig_lower(self, ordered)

    def patched_drain(self, tick_clock, wait_clock):
        sems_alloc = list(self.sems.allocated().values())
        carrier = self.nc.sync.wait_ge(sems_alloc[0], 0)
        wait_clock.add_sem_waits(
            carrier.ins, tile.ScopedClock({None: tick_clock.global_clock})
        )
        waits = list(carrier.ins.sync_info.on_wait)
        carrier.ins.sync_info.on_wait.clear()
        for sw in waits[:2]:
            carrier.ins.sync_info.on_wait.append(sw)
        for i in range(2, len(waits), 2):
            c = self.nc.sync.wait_ge(sems_alloc[0], 0)
            c.ins.sync_info.on_wait.clear()
            for sw in waits[i : i + 2]:
                c.ins.sync_info.on_wait.append(sw)
        self.nc.sync.drain(fusable=False)
        self.nc.all_engine_barrier()
        popped = self.nc._tile_sem_poison_stack.pop()
        assert popped is self._sem_poison
        self.nc.clear_and_free_semaphores(sems_alloc)
        self.nc.all_engine_barrier()

    tile.TileContext._lower_ordered_insts = patched_lower
    tile.TileContext._drain_and_barrier = patched_drain

    # NTFF profile hook shim (missing antenv.axon_hooks in this image)
    _hook = {}
    mod = types.ModuleType("antenv.axon_hooks")
    mod.set_axon_ntff_profile_hook = lambda h: _hook.update(hook=h)
    mod.get_axon_ntff_profile_hook = lambda: _hook.get("hook")
    sys.modules["antenv.axon_hooks"] = mod
    try:
        import antenv

        antenv.axon_hooks = mod
        from trn_agent_boot.trn_boot import _ntff_profile_via_ctypes

        mod.set_axon_ntff_profile_hook(
            _ntff_profile_via_ctypes("/opt/axon/libaxon_pjrt.so")
        )
    except Exception:
        pass
    _CACHE["compat"] = True


# ---------------------------------------------------------- preprocessing ---
class Schedule:
    pass


def _round_up(v, m):
    return (v + m - 1) // m * m


def _preprocess(n, edge_index):
    """Build the uniform cross-core schedule + per-core data streams.

    Layout: for each superblock s (SB_N dst blocks), for each src bank k,
    a segment holding each block's edges padded to the max count over
    cores, chunked into 128-slot units.  Chunks may span block boundaries;
    each (chunk, block) pair is one matmul instance with its own
    indicator column stream.
    """
    shard = n // NCORES
    nblk = (shard + P - 1) // P
    nbanks = 4
    qs = shard // nbanks
    bank_rows = (n + nbanks - 1) // nbanks
    assert bank_rows <= 32767
    n_sb = (nblk + SB_N - 1) // SB_N

    src = edge_index[0].astype(np.int64)
    dst = edge_index[1].astype(np.int64)
    e = src.shape[0]
    deg = np.bincount(dst, minlength=n).astype(np.float64) + 1.0
    dinv = (1.0 / np.sqrt(deg)).astype(np.float32)

    core = dst // shard
    dl = dst - core * shard
    blk = dl // P
    dstloc = (dl % P).astype(np.int32)
    bank = src // bank_rows
    bidx = (src % bank_rows).astype(np.int32)

    # per (core, blk, bank) counts -> per-block max over cores
    cnt = np.bincount(
        (core * nblk + blk) * nbanks + bank, minlength=NCORES * nblk * nbanks
    ).reshape(NCORES, nblk, nbanks)
    mx = cnt.max(axis=0)  # [nblk, nbanks]

    # ---- compile-time program ----------------------------------------
    # gathers: (c16, nidx, bank, sb, i0, ninst) ; instances: (g, j, b, stop)
    sbs = []
    tot_slots = 0
    ninst_tot = 0
    gathers_all = []
    # chunk slot ranges per (sb, bank): block boundaries inside the segment
    seg_info = {}  # (s, k) -> (slot_base, [(b, off, mxcnt)...], nch_padded)
    for s in range(n_sb):
        blocks = list(range(s * SB_N, min((s + 1) * SB_N, nblk)))
        sb = {"blocks": blocks, "gathers": [], "instances": []}
        # last accumulate instance per block for stop flags
        for k in range(nbanks):
            seg_blocks = []
            off = 0
            for b in blocks:
                seg_blocks.append((b, off, int(mx[b, k])))
                off += int(mx[b, k])
            nseg = off
            nch_real = (nseg + P - 1) // P
            # split into gathers of <= GMAX chunks, pad each to mult of 4
            c0 = 0
            gath_list = []
            while c0 < nch_real:
                gn_real = min(GMAX_CHUNKS, nch_real - c0)
                gath_list.append((c0, gn_real, gn_real))
                c0 += gn_real
            seg_info[(s, k)] = (tot_slots, seg_blocks, gath_list)
            for (c0g, gn_real, gn) in gath_list:
                # instances: blocks overlapping chunks [c0g, c0g+gn_real)
                insts = []
                for j in range(gn_real):
                    lo = (c0g + j) * P
                    hi = lo + P
                    for (b, boff, bmx) in seg_blocks:
                        if boff < hi and boff + bmx > lo:
                            insts.append((j, b))
                g = {
                    "c16": tot_slots // 16,
                    "nidx": gn * P,
                    "bank": k,
                    "i0": ninst_tot,
                    "insts": insts,
                    "gi": len(gathers_all),
                }
                ninst_tot += len(insts)
                tot_slots += gn * P
                sb["gathers"].append(g)
                gathers_all.append(g)
        # stop flags: last instance of each block within this sb
        last = {}
        for g in sb["gathers"]:
            for idx, (j, b) in enumerate(g["insts"]):
                last[b] = (id(g), idx)
        for g in sb["gathers"]:
            g["stops"] = [
                last.get(b) == (id(g), idx) for idx, (j, b) in enumerate(g["insts"])
            ]
        sbs.append(sb)

    # ---- per-core streams ---------------------------------------------
    idx_stream = np.zeros((NCORES, 16, tot_slots // 16), np.int16)
    dstloc_s = np.full((NCORES, P, ninst_tot), -1.0, ml_dtypes.bfloat16)
    ngath = len(gathers_all)
    gcnt = np.zeros((NCORES, ngath), np.int32)

    order = np.lexsort((bidx, blk, bank, core))
    so_core = core[order]
    so_blk = blk[order]
    so_bank = bank[order]
    so_bidx = bidx[order]
    so_dstloc = dstloc[order]
    ptr = np.searchsorted(so_core, np.arange(NCORES + 1))

    for c in range(NCORES):
        lo, hi = ptr[c], ptr[c + 1]
        cblk = so_blk[lo:hi]
        cbank = so_bank[lo:hi]
        cbidx = so_bidx[lo:hi]
        cdl = so_dstloc[lo:hi]
        # run starts per (bank, blk): sorted by (bank, blk)
        key = cbank * nblk + cblk
        idx_flat = np.full(tot_slots, -1, np.int32)
        dl_flat = np.full(tot_slots, -1, np.int32)
        blk_flat = np.full(tot_slots, -2, np.int32)
        real = np.zeros(tot_slots, bool)
        for s in range(n_sb):
            for k in range(nbanks):
                slot_base, seg_blocks, gath_list = seg_info[(s, k)]
                for (b, boff, bmx) in seg_blocks:
                    want = k * nblk + b
                    p0 = np.searchsorted(key, want, "left")
                    p1 = np.searchsorted(key, want, "right")
                    cn = p1 - p0
                    assert cn <= bmx
                    sl = slot_base + boff
                    idx_flat[sl : sl + cn] = cbidx[p0:p1]
                    dl_flat[sl : sl + cn] = cdl[p0:p1]
                    blk_flat[sl : sl + cn] = b
                    real[sl : sl + cn] = True
        # per-gather: pads before the last real slot gather row 0 (harmless),
        # pads after are trailing -1 (skipped by Q7 desc-gen; count goes in
        # the per-core num_idxs register).
        for gi, g in enumerate(gathers_all):
            c16, nidx = g["c16"], g["nidx"]
            w0 = c16 * 16
            rw = real[w0 : w0 + nidx]
            nz = np.nonzero(rw)[0]
            last = int(nz[-1]) + 1 if len(nz) else 0
            gcnt[c, gi] = last
            w = idx_flat[w0 : w0 + nidx]
            w[:last][~rw[:last]] = 0
            w[last:] = -1
            idx_stream[c][:, c16 : c16 + nidx // 16] = (
                w.reshape(nidx // 16, 16).T.astype(np.int16)
            )
            # instance dstloc columns
            for ii, (j, b) in enumerate(g["insts"]):
                cslots = slice(c16 * 16 + j * P, c16 * 16 + (j + 1) * P)
                m = blk_flat[cslots] == b
                col = np.where(m, dl_flat[cslots], -1).astype(np.float32)
                dstloc_s[c][:, g["i0"] + ii] = col.astype(ml_dtypes.bfloat16)

    sch = Schedule()
    sch.n, sch.e, sch.shard, sch.nblk, sch.nbanks = n, e, shard, nblk, nbanks
    sch.bank_rows, sch.n_sb, sch.qs = bank_rows, n_sb, qs
    sch.tot_slots = tot_slots
    sch.ninst = ninst_tot
    sch.ngath = ngath
    sch.gcnt = gcnt
    sch.sbs = sbs
    sch.dinv = dinv
    sch.idx_stream = np.tile(idx_stream, (1, 8, 1))  # replicate to 128 parts
    sch.dstloc_s = dstloc_s
    return sch


# ----------------------------------------------------------------- build ---
def _build(sch, in_dim, hid, out_dim, bias_zero):
    import concourse.mybir as mybir
    import concourse.tile as tile
    from concourse import bacc

    bf16 = mybir.dt.bfloat16
    f32 = mybir.dt.float32
    shard, nblk, nbanks = sch.shard, sch.nblk, sch.nbanks
    n_sb = sch.n_sb
    n = sch.n

    nc = bacc.Bacc(num_swdge_queues=NQUEUES, dynamic_dma_scratch_size=32768)

    xT = nc.declare_dram_parameter("xT", [in_dim, shard], bf16, isOutput=False)
    idxs = nc.declare_dram_parameter(
        "idxs", [P, sch.tot_slots // 16], mybir.dt.int16, isOutput=False
    )
    dstloc = nc.declare_dram_parameter("dstloc", [P, sch.ninst], bf16, isOutput=False)
    gcnt_in = nc.declare_dram_parameter(
        "gcnt", [1, sch.ngath], mybir.dt.int32, isOutput=False
    )
    iotar_in = nc.declare_dram_parameter(
        "iotar", [P, IOTAR_K * P], bf16, isOutput=False
    )
    ident_in = nc.declare_dram_parameter("ident", [P, P], bf16, isOutput=False)
    dinvb = nc.declare_dram_parameter("dinvb", [P, nblk], f32, isOutput=False)
    dinvsq = nc.declare_dram_parameter("dinvsq", [P, nblk], f32, isOutput=False)
    w1 = nc.declare_dram_parameter("W1", [in_dim, hid], bf16, isOutput=False)
    w2 = nc.declare_dram_parameter("W2", [hid, out_dim], bf16, isOutput=False)
    if not bias_zero:
        b1r = nc.declare_dram_parameter("b1r", [1, hid], bf16, isOutput=False)
        b2r = nc.declare_dram_parameter("b2r", [1, out_dim], bf16, isOutput=False)
        rdinvr = nc.declare_dram_parameter(
            "rdinvr", [1, nblk * P], bf16, isOutput=False
        )
    out_ext = nc.declare_dram_parameter("out", [shard, out_dim], f32, isOutput=True)

    hloc = nc.dram_tensor("hloc", [shard, hid], bf16)
    hfull = nc.dram_tensor("hfull", [n, hid], bf16, addr_space="Shared")
    h2loc = nc.dram_tensor("h2loc", [shard, P], bf16)
    h2full = nc.dram_tensor("h2full", [n, P], bf16, addr_space="Shared")

    kin = in_dim // P  # contraction tiles for layer-1 matmul

    with tile.TileContext(nc) as tc:
        with (
            tc.tile_pool(name="const", bufs=1) as cpool,
            tc.tile_pool(name="xload", bufs=2) as xpool,
            tc.tile_pool(name="idx", bufs=8) as ipool,
            tc.tile_pool(name="gath", bufs=7) as gpool,
            tc.tile_pool(name="sind", bufs=3) as spool,
            tc.tile_pool(name="blk", bufs=6) as bpool,
            tc.tile_pool(name="psagg", bufs=1, space="PSUM") as psagg,
            tc.tile_pool(name="psh2", bufs=1, space="PSUM") as psh2,
        ):
            import contextlib

            regstack = contextlib.ExitStack()
            nreg = regstack.enter_context(nc.gpsimd.register("nreg"))
            # ---- constants into SBUF
            ident_sb = cpool.tile([P, P], bf16, tag="ident")
            nc.sync.dma_start(out=ident_sb[:], in_=ident_in[:])
            w1_t = [
                cpool.tile([P, hid], bf16, tag=f"w1_{k}", name=f"w1t{k}")
                for k in range(kin)
            ]
            for k in range(kin):
                nc.sync.dma_start(out=w1_t[k][:], in_=w1[k * P : (k + 1) * P, :])
            w2_sb = cpool.tile([hid, out_dim], bf16, tag="w2")
            nc.sync.dma_start(out=w2_sb[:], in_=w2[:])
            dinvb_sb = cpool.tile([P, nblk], f32, tag="dinvb")
            nc.sync.dma_start(out=dinvb_sb[:], in_=dinvb[:])
            dinvsq_sb = cpool.tile([P, nblk], f32, tag="dinvsq")
            nc.sync.dma_start(out=dinvsq_sb[:], in_=dinvsq[:])
            dstloc_sb = cpool.tile([P, sch.ninst], bf16, tag="dstloc")
            nc.sync.dma_start(out=dstloc_sb[:], in_=dstloc[:])
            gcnt_sb = cpool.tile([1, sch.ngath], mybir.dt.int32, tag="gcnt")
            nc.sync.dma_start(out=gcnt_sb[:], in_=gcnt_in[:])
            iotar_sb = cpool.tile([P, IOTAR_K * P], bf16, tag="iotar")
            nc.sync.dma_start(out=iotar_sb[:], in_=iotar_in[:])
            if not bias_zero:
                b1_sb = cpool.tile([1, hid], bf16, tag="b1r")
                nc.sync.dma_start(out=b1_sb[:], in_=b1r[:])
                b2_sb = cpool.tile([1, out_dim], bf16, tag="b2r")
                nc.sync.dma_start(out=b2_sb[:], in_=b2r[:])
                rdinv_sb = cpool.tile([1, nblk * P], bf16, tag="rdinvr")
                nc.sync.dma_start(out=rdinv_sb[:], in_=rdinvr[:])
            # SBUF-resident tables for self-loop adds
            hres = cpool.tile([P, nblk * hid], bf16, tag="hres")
            h2self = cpool.tile([P, nblk * out_dim], bf16, tag="h2self")

            # PSUM: matmul start=True zeroes the whole 2KB bank ("zero
            # region"), so each accumulation slot owns a full bank.
            psagg_t = [
                psagg.tile([P, 4 * P], f32, tag=f"agg{i}", name=f"aggbank{i}")
                for i in range(SB_N)
            ]

            def slot_ap(si, w):
                return psagg_t[si][:, :w]

            # ---- h~ = dinv * (x @ W1), shard-local, bf16
            XGRP = 8
            for g0 in range(0, nblk, XGRP):
                g1 = min(g0 + XGRP, nblk)
                c0, c1 = g0 * P, min(g1 * P, shard)
                xt = [
                    xpool.tile([P, XGRP * P], bf16, tag=f"xt{k}", name=f"xt{k}")
                    for k in range(kin)
                ]
                for k in range(kin):
                    nc.sync.dma_start(
                        out=xt[k][:, : c1 - c0], in_=xT[k * P : (k + 1) * P, c0:c1]
                    )
                for b in range(g0, g1):
                    m = min(P, shard - b * P)
                    hp = psagg_t[b % 2]
                    for k in range(kin):
                        nc.tensor.matmul(
                            out=hp[:m, :hid],
                            lhsT=xt[k][:, b * P - c0 : b * P - c0 + m],
                            rhs=w1_t[k][:],
                            start=(k == 0),
                            stop=(k == kin - 1),
                        )
                    nc.scalar.activation(
                        out=hres[:m, b * hid : b * hid + hid],
                        in_=hp[:m, :hid],
                        func=mybir.ActivationFunctionType.Copy,
                        scale=dinvb_sb[:m, b : b + 1],
                    )
                    nc.sync.dma_start(
                        out=hloc[b * P : b * P + m, :],
                        in_=hres[:m, b * hid : b * hid + hid],
                    )

            nc.gpsimd.collective_compute(
                "AllGather",
                mybir.AluOpType.bypass,
                ins=[hloc[:]],
                outs=[hfull[:]],
                replica_groups=[list(range(NCORES))],
            )

            gq = [0]

            def run_layer(layer):
                table = hfull if layer == 1 else h2full
                w = P if layer == 1 else out_dim
                for s in range(n_sb):
                    sb = sch.sbs[s]
                    blocks = sb["blocks"]
                    slot_of = {b: i for i, b in enumerate(blocks)}
                    # self-loop add opens each block's accumulation
                    for b in blocks:
                        m = min(P, shard - b * P)
                        si = slot_of[b]
                        if layer == 1:
                            nc.tensor.matmul(
                                out=slot_ap(si, P),
                                lhsT=hres[:m, b * hid : b * hid + hid],
                                rhs=ident_sb[:m, :],
                                start=True,
                                stop=False,
                            )
                        else:
                            nc.tensor.matmul(
                                out=slot_ap(si, out_dim),
                                lhsT=ident_sb[:m, :],
                                rhs=h2self[:m, b * out_dim : (b + 1) * out_dim],
                                start=True,
                                stop=False,
                            )
                        if not bias_zero:
                            # rank-1 bias: b1 (x) 1/dinv  /  b2/dinv
                            if layer == 1:
                                nc.tensor.matmul(
                                    out=slot_ap(si, P),
                                    lhsT=b1_sb[:, :],
                                    rhs=rdinv_sb[:, b * P : (b + 1) * P],
                                    start=False,
                                    stop=False,
                                )
                            else:
                                nc.tensor.matmul(
                                    out=slot_ap(si, out_dim),
                                    lhsT=rdinv_sb[:, b * P : (b + 1) * P],
                                    rhs=b2_sb[:, :],
                                    start=False,
                                    stop=False,
                                )
                    for g in sb["gathers"]:
                        c16, nidx, k = g["c16"], g["nidx"], g["bank"]
                        gi = g["gi"]
                        nch = nidx // P
                        ninst_g = len(g["insts"])
                        it = ipool.tile(
                            [P, GMAX_CHUNKS * 8], mybir.dt.int16, tag="it"
                        )
                        nc.sync.dma_start(
                            out=it[:, : nidx // 16],
                            in_=idxs[:, c16 : c16 + nidx // 16],
                        )
                        gt = gpool.tile([P, GMAX_CHUNKS, P], bf16, tag="gt")
                        if gq[0] < 7:
                            # first use of each pool buffer: clear stale SBUF
                            # so trimmed (unwritten) slots stay finite
                            nc.vector.memset(gt[:], 0.0)
                        r0 = k * sch.bank_rows
                        r1 = min(r0 + sch.bank_rows, n)
                        nc.gpsimd.reg_load(nreg, gcnt_sb[:1, gi : gi + 1])
                        nc.gpsimd.dma_gather(
                            out_ap=gt[:, :nch, :],
                            in_ap=table[r0:r1, :],
                            idxs_ap=it[:, : nidx // 16],
                            num_idxs=nidx,
                            num_idxs_reg=nreg,
                            elem_size=P,
                            single_packet=False,
                            queue_num=gq[0] % NQUEUES,
                        )
                        gq[0] += 1
                        # batched indicator build for all instances
                        sbig = spool.tile([P, IOTAR_K, P], bf16, tag="sind")
                        nc.vector.tensor_tensor(
                            out=sbig[:, :ninst_g, :],
                            in0=iotar_sb[:, : ninst_g * P].rearrange(
                                "p (k f) -> p k f", k=ninst_g
                            ),
                            in1=dstloc_sb[
                                :, g["i0"] : g["i0"] + ninst_g
                            ].to_broadcast([P, ninst_g, P]),
                            op=mybir.AluOpType.is_equal,
                        )
                        for ii, (j, b) in enumerate(g["insts"]):
                            si = slot_of[b]
                            sp = g["stops"][ii]
                            if layer == 1:
                                nc.tensor.matmul(
                                    out=slot_ap(si, P),
                                    lhsT=gt[:, j, :],
                                    rhs=sbig[:, ii, :],
                                    start=False,
                                    stop=sp,
                                )
                            else:
                                nc.tensor.matmul(
                                    out=slot_ap(si, out_dim),
                                    lhsT=sbig[:, ii, :],
                                    rhs=gt[:, j, :out_dim],
                                    start=False,
                                    stop=sp,
                                )
                    # ---- block epilogues (Scalar + TensorE only)
                    for b in blocks:
                        m = min(P, shard - b * P)
                        si = slot_of[b]
                        if layer == 1:
                            o1 = bpool.tile([P, P], bf16, tag="o1")
                            nc.scalar.activation(
                                out=o1[:],
                                in_=slot_ap(si, P),
                                func=mybir.ActivationFunctionType.Relu,
                            )
                            h2p = psh2.tile([P, out_dim], f32, tag="h2p")
                            nc.tensor.matmul(
                                out=h2p[:m, :],
                                lhsT=o1[:, :m],
                                rhs=w2_sb[:],
                                start=True,
                                stop=True,
                            )
                            h2s = bpool.tile([P, P], bf16, tag="h2s")
                            nc.vector.memset(h2s[:, out_dim:], 0.0)
                            nc.scalar.activation(
                                out=h2s[:m, :out_dim],
                                in_=h2p[:m, :],
                                func=mybir.ActivationFunctionType.Copy,
                                scale=dinvsq_sb[:m, b : b + 1],
                            )
                            nc.scalar.activation(
                                out=h2self[:m, b * out_dim : (b + 1) * out_dim],
                                in_=h2p[:m, :],
                                func=mybir.ActivationFunctionType.Copy,
                                scale=dinvsq_sb[:m, b : b + 1],
                            )
                            nc.sync.dma_start(
                                out=h2loc[b * P : b * P + m, :], in_=h2s[:m, :]
                            )
                        else:
                            ob = bpool.tile([P, out_dim], f32, tag="ob")
                            nc.scalar.activation(
                                out=ob[:m, :],
                                in_=slot_ap(si, out_dim)[:m, :],
                                func=mybir.ActivationFunctionType.Copy,
                                scale=dinvb_sb[:m, b : b + 1],
                            )
                            nc.sync.dma_start(
                                out=out_ext[b * P : b * P + m, :], in_=ob[:m, :]
                            )

            run_layer(1)
            nc.gpsimd.collective_compute(
                "AllGather",
                mybir.AluOpType.bypass,
                ins=[h2loc[:]],
                outs=[h2full[:]],
                replica_groups=[list(range(NCORES))],
            )
            run_layer(2)
            regstack.close()

    nc.compile()
    return nc


# ---------------------------------------------------------------- kernel ---
def _make_in_maps(sch, x, W1, b1v, W2, b2v, bias_zero):
    hid = W1.shape[1]
    out_dim = W2.shape[1]
    shard = sch.shard
    nblk = sch.nblk
    bf = ml_dtypes.bfloat16
    in_maps = []
    w1b = W1.astype(bf)
    w2b = W2.astype(bf)
    iotar = np.tile(np.arange(P, dtype=np.float32), (P, IOTAR_K)).astype(bf)
    ident = np.eye(P, dtype=np.float32).astype(bf)
    for c in range(NCORES):
        xs = np.ascontiguousarray(x[c * shard : (c + 1) * shard].astype(bf).T)
        dv = sch.dinv[c * shard : (c + 1) * shard].astype(np.float64)
        full = np.zeros(nblk * P, np.float64)
        full[:shard] = dv
        cols = np.ascontiguousarray(full.reshape(nblk, P).T)
        m = {
            "xT": xs,
            "idxs": sch.idx_stream[c],
            "dstloc": sch.dstloc_s[c],
            "gcnt": sch.gcnt[c].reshape(1, -1),
            "iotar": iotar,
            "ident": ident,
            "dinvb": cols.astype(np.float32),
            "dinvsq": (cols**2).astype(np.float32),
            "W1": w1b,
            "W2": w2b,
        }
        if not bias_zero:
            rd = np.zeros(nblk * P, np.float64)
            rd[:shard] = 1.0 / dv
            m["b1r"] = b1v.reshape(1, hid).astype(bf)
            m["b2r"] = b2v.reshape(1, out_dim).astype(bf)
            m["rdinvr"] = rd.reshape(1, nblk * P).astype(bf)
        in_maps.append(m)
    return in_maps


def _get_compiled(n, e, edge_index, in_dim, hid, out_dim, bias_zero):
    key = ("nc", n, e, bias_zero)
    if key not in _CACHE:
        sch = _preprocess(n, edge_index)
        _CACHE[("sched", n, e)] = sch
        _CACHE[key] = _build(sch, in_dim, hid, out_dim, bias_zero)
    return _CACHE[("sched", n, e)], _CACHE[key]


def kernel(x, edge_index, W1, b1, W2, b2):
    _install_compat()
    from concourse.bass_utils import run_bass_kernel_spmd

    x = np.asarray(x)
    edge_index = np.asarray(edge_index)
    W1 = np.asarray(W1, np.float32)
    b1v = np.asarray(b1, np.float32)
    W2 = np.asarray(W2, np.float32)
    b2v = np.asarray(b2, np.float32)
    n, in_dim = x.shape
    hid = W1.shape[1]
    out_dim = W2.shape[1]
    bias_zero = bool(np.all(b1v == 0) and np.all(b2v == 0))

    sch, nc = _get_compiled(
        n, edge_index.shape[1], edge_index, in_dim, hid, out_dim, bias_zero
    )
    in_maps = _make_in_maps(sch, x, W1, b1v, W2, b2v, bias_zero)
    import os

    trace = bool(os.environ.get("GCN_TRACE"))
    res = run_bass_kernel_spmd(
        nc, in_maps, core_ids=list(range(NCORES)), trace=trace
    )
    global LAST_EXEC_NS
    LAST_EXEC_NS = res.exec_time_ns
    return np.concatenate([res.results[c]["out"] for c in range(NCORES)], axis=0)


LAST_EXEC_NS = None


# revision 52
# speedup vs baseline: 3.2607x; 1.3219x over previous
"""2-layer GCN (GCNConv x2) on 8 Trainium2 NeuronCores.

Strategy (dst-sharded, edge-partitioned by destination):
- Each core owns N/8 destination nodes and the edges pointing at them.
- h~ = dinv * (x @ W1) computed per-shard, AllGathered to a full bf16 table.
- Per-edge messages fetched with dma_gather (4 SWDGE queues round-robin);
  scatter-add done as one-hot-indicator matmuls accumulating in PSUM.
- Q7 descriptor generation is the critical resource, so the schedule
  minimizes gathered slots: no self-loop edges (self term is an
  identity-matmul add of SBUF-resident rows), per-(superblock, bank)
  chunking with per-block max-core slot packing, trailing -1 indices so
  each core only generates descriptors for its real edge count.
- Epilogues run on Scalar/TensorE only (relu(dinv*x) = dinv*relu(x));
  DVE just builds indicator matrices (batched is_equal per gather).
"""
import sys
import types

import numpy as np
import ml_dtypes

P = 128
NCORES = 8
GMAX_CHUNKS = 40  # max chunks (128 idxs each) per dma_gather
SB_N = 7  # dst blocks per super-block = PSUM accumulation banks
NQUEUES = 4
IOTAR_K = 48  # max instances per batched is_eq

_CACHE = {}


# ---------------------------------------------------------------- compat ---
def _install_compat():
    """Patches for this axon/walrus stack (drain waits, per-inst wait caps,
    NTFF shim). Idempotent."""
    if _CACHE.get("compat"):
        return
    import concourse.tile as tile
    import concourse.mybir as mybir

    _ev = [0]

    def _split_inst_waits(ordered):
        for _bb, insts in ordered.items():
            out = []
            for inst in insts:
                si = getattr(inst, "sync_info", None)
                if si is not None and si.on_wait is not None and len(si.on_wait) > 1:
                    waits = list(si.on_wait)
                    excess, keep = waits[:-1], waits[-1:]
                    si.on_wait.clear()
                    for sw in keep:
                        si.on_wait.append(sw)
                    for i in range(0, len(excess), 2):
                        _ev[0] += 1
                        ev = mybir.InstEventSemaphore(
                            name=f"evsplit-{_ev[0]}", ins=[], outs=[]
                        )
                        ev.engine = inst.engine
                        ev.sync_info = mybir.SyncInfo(
                            on_wait=excess[i : i + 2], on_update=[]
                        )
                        out.append(ev)
                out.append(inst)
            insts[:] = out

    orig_lower = tile.TileContext._lower_ordered_insts

    def patched_lower(self, ordered):
        _split_inst_waits(ordered)
        return orig_lower(self, ordered)

    def patched_drain(self, tick_clock, wait_clock):
        sems_alloc = list(self.sems.allocated().values())
        carrier = self.nc.sync.wait_ge(sems_alloc[0], 0)
        wait_clock.add_sem_waits(
            carrier.ins, tile.ScopedClock({None: tick_clock.global_clock})
        )
        waits = list(carrier.ins.sync_info.on_wait)
        carrier.ins.sync_info.on_wait.clear()
        for sw in waits[:2]:
            carrier.ins.sync_info.on_wait.append(sw)
        for i in range(2, len(waits), 2):
            c = self.nc.sync.wait_ge(sems_alloc[0], 0)
            c.ins.sync_info.on_wait.clear()
            for sw in waits[i : i + 2]:
                c.ins.sync_info.on_wait.append(sw)
        self.nc.sync.drain(fusable=False)
        self.nc.all_engine_barrier()
        popped = self.nc._tile_sem_poison_stack.pop()
        assert popped is self._sem_poison
        self.nc.clear_and_free_semaphores(sems_alloc)
        self.nc.all_engine_barrier()

    tile.TileContext._lower_ordered_insts = patched_lower
    tile.TileContext._drain_and_barrier = patched_drain

    # NTFF profile hook shim (missing antenv.axon_hooks in this image)
    _hook = {}
    mod = types.ModuleType("antenv.axon_hooks")
    mod.set_axon_ntff_profile_hook = lambda h: _hook.update(hook=h)
    mod.get_axon_ntff_profile_hook = lambda: _hook.get("hook")
    sys.modules["antenv.axon_hooks"] = mod
    try:
        import antenv

        antenv.axon_hooks = mod
        from trn_agent_boot.trn_boot import _ntff_profile_via_ctypes

        mod.set_axon_ntff_profile_hook(
            _ntff_profile_via_ctypes("/opt/axon/libaxon_pjrt.so")
        )
    except Exception:
        pass
    _CACHE["compat"] = True


# ---------------------------------------------------------- preprocessing ---
class Schedule:
    pass


def _round_up(v, m):
    return (v + m - 1) // m * m


def _preprocess(n, edge_index):
    """Build the uniform cross-core schedule + per-core data streams.

    Layout: for each superblock s (SB_N dst blocks), for each src bank k,
    a segment holding each block's edges padded to the max count over
    cores, chunked into 128-slot units.  Chunks may span block boundaries;
    each (chunk, block) pair is one matmul instance with its own
    indicator column stream.
    """
    shard = n // NCORES
    nblk = (shard + P - 1) // P
    nbanks = 4
    qs = shard // nbanks
    bank_rows = (n + nbanks - 1) // nbanks
    assert bank_rows <= 32767
    n_sb = (nblk + SB_N - 1) // SB_N

    src = edge_index[0].astype(np.int64)
    dst = edge_index[1].astype(np.int64)
    e = src.shape[0]
    deg = np.bincount(dst, minlength=n).astype(np.float64) + 1.0
    dinv = (1.0 / np.sqrt(deg)).astype(np.float32)

    core = dst // shard
    dl = dst - core * shard
    blk = dl // P
    dstloc = (dl % P).astype(np.int32)
    bank = src // bank_rows
    bidx = (src % bank_rows).astype(np.int32)

    # per (core, blk, bank) counts -> per-block max over cores
    cnt = np.bincount(
        (core * nblk + blk) * nbanks + bank, minlength=NCORES * nblk * nbanks
    ).reshape(NCORES, nblk, nbanks)
    mx = cnt.max(axis=0)  # [nblk, nbanks]

    # ---- compile-time program ----------------------------------------
    # gathers: (c16, nidx, bank, sb, i0, ninst) ; instances: (g, j, b, stop)
    sbs = []
    tot_slots = 0
    ninst_tot = 0
    gathers_all = []
    # chunk slot ranges per (sb, bank): block boundaries inside the segment
    seg_info = {}  # (s, k) -> (slot_base, [(b, off, mxcnt)...], nch_padded)
    for s in range(n_sb):
        blocks = list(range(s * SB_N, min((s + 1) * SB_N, nblk)))
        sb = {"blocks": blocks, "gathers": [], "instances": []}
        # last accumulate instance per block for stop flags
        for k in range(nbanks):
            seg_blocks = []
            off = 0
            for b in blocks:
                seg_blocks.append((b, off, int(mx[b, k])))
                off += int(mx[b, k])
            nseg = off
            nch_real = (nseg + P - 1) // P
            # split into gathers of <= GMAX chunks, pad each to mult of 4
            c0 = 0
            gath_list = []
            while c0 < nch_real:
                gn_real = min(GMAX_CHUNKS, nch_real - c0)
                gath_list.append((c0, gn_real, gn_real))
                c0 += gn_real
            seg_info[(s, k)] = (tot_slots, seg_blocks, gath_list)
            for (c0g, gn_real, gn) in gath_list:
                # instances: blocks overlapping chunks [c0g, c0g+gn_real)
                insts = []
                for j in range(gn_real):
                    lo = (c0g + j) * P
                    hi = lo + P
                    for (b, boff, bmx) in seg_blocks:
                        if boff < hi and boff + bmx > lo:
                            insts.append((j, b))
                g = {
                    "c16": tot_slots // 16,
                    "nidx": gn * P,
                    "bank": k,
                    "i0": ninst_tot,
                    "insts": insts,
                    "gi": len(gathers_all),
                }
                ninst_tot += len(insts)
                tot_slots += gn * P
                sb["gathers"].append(g)
                gathers_all.append(g)
        # stop flags: last instance of each block within this sb
        last = {}
        for g in sb["gathers"]:
            for idx, (j, b) in enumerate(g["insts"]):
                last[b] = (id(g), idx)
        for g in sb["gathers"]:
            g["stops"] = [
                last.get(b) == (id(g), idx) for idx, (j, b) in enumerate(g["insts"])
            ]
        sbs.append(sb)

    # ---- per-core streams ---------------------------------------------
    idx_stream = np.zeros((NCORES, 16, tot_slots // 16), np.int16)
    dstloc_s = np.full((NCORES, P, ninst_tot), -1.0, ml_dtypes.bfloat16)
    ngath = len(gathers_all)
    gcnt = np.zeros((NCORES, ngath), np.int32)

    order = np.lexsort((bidx, blk, bank, core))
    so_core = core[order]
    so_blk = blk[order]
    so_bank = bank[order]
    so_bidx = bidx[order]
    so_dstloc = dstloc[order]
    ptr = np.searchsorted(so_core, np.arange(NCORES + 1))

    for c in range(NCORES):
        lo, hi = ptr[c], ptr[c + 1]
        cblk = so_blk[lo:hi]
        cbank = so_bank[lo:hi]
        cbidx = so_bidx[lo:hi]
        cdl = so_dstloc[lo:hi]
        # run starts per (bank, blk): sorted by (bank, blk)
        key = cbank * nblk + cblk
        idx_flat = np.full(tot_slots, -1, np.int32)
        dl_flat = np.full(tot_slots, -1, np.int32)
        blk_flat = np.full(tot_slots, -2, np.int32)
        real = np.zeros(tot_slots, bool)
        for s in range(n_sb):
            for k in range(nbanks):
                slot_base, seg_blocks, gath_list = seg_info[(s, k)]
                for (b, boff, bmx) in seg_blocks:
                    want = k * nblk + b
                    p0 = np.searchsorted(key, want, "left")
                    p1 = np.searchsorted(key, want, "right")
                    cn = p1 - p0
                    assert cn <= bmx
                    sl = slot_base + boff
                    idx_flat[sl : sl + cn] = cbidx[p0:p1]
                    dl_flat[sl : sl + cn] = cdl[p0:p1]
                    blk_flat[sl : sl + cn] = b
                    real[sl : sl + cn] = True
        # per-gather: pads before the last real slot gather row 0 (harmless),
        # pads after are trailing -1 (skipped by Q7 desc-gen; count goes in
        # the per-core num_idxs register).
        for gi, g in enumerate(gathers_all):
            c16, nidx = g["c16"], g["nidx"]
            w0 = c16 * 16
            rw = real[w0 : w0 + nidx]
            nz = np.nonzero(rw)[0]
            last = int(nz[-1]) + 1 if len(nz) else 0
            gcnt[c, gi] = last
            w = idx_flat[w0 : w0 + nidx]
            w[:last][~rw[:last]] = 0
            w[last:] = -1
            idx_stream[c][:, c16 : c16 + nidx // 16] = (
                w.reshape(nidx // 16, 16).T.astype(np.int16)
            )
            # instance dstloc columns
            for ii, (j, b) in enumerate(g["insts"]):
                cslots = slice(c16 * 16 + j * P, c16 * 16 + (j + 1) * P)
                m = blk_flat[cslots] == b
                col = np.where(m, dl_flat[cslots], -1).astype(np.float32)
                dstloc_s[c][:, g["i0"] + ii] = col.astype(ml_dtypes.bfloat16)

    sch = Schedule()
    sch.n, sch.e, sch.shard, sch.nblk, sch.nbanks = n, e, shard, nblk, nbanks
    sch.bank_rows, sch.n_sb, sch.qs = bank_rows, n_sb, qs
    sch.tot_slots = tot_slots
    sch.ninst = ninst_tot
    sch.ngath = ngath
    sch.gcnt = gcnt
    sch.sbs = sbs
    sch.dinv = dinv
    sch.idx_stream = np.tile(idx_stream, (1, 8, 1))  # replicate to 128 parts
    sch.dstloc_s = dstloc_s
    return sch


# ----------------------------------------------------------------- build ---
def _build(sch, in_dim, hid, out_dim, bias_zero):
    import concourse.mybir as mybir
    import concourse.tile as tile
    from concourse import bacc

    bf16 = mybir.dt.bfloat16
    f32 = mybir.dt.float32
    shard, nblk, nbanks = sch.shard, sch.nblk, sch.nbanks
    n_sb = sch.n_sb
    n = sch.n

    nc = bacc.Bacc(num_swdge_queues=NQUEUES, dynamic_dma_scratch_size=32768)

    xT = nc.declare_dram_parameter("xT", [in_dim, shard], bf16, isOutput=False)
    idxs = nc.declare_dram_parameter(
        "idxs", [P, sch.tot_slots // 16], mybir.dt.int16, isOutput=False
    )
    dstloc = nc.declare_dram_parameter("dstloc", [P, sch.ninst], bf16, isOutput=False)
    gcnt_in = nc.declare_dram_parameter(
        "gcnt", [1, sch.ngath], mybir.dt.int32, isOutput=False
    )
    iotar_in = nc.declare_dram_parameter(
        "iotar", [P, IOTAR_K * P], bf16, isOutput=False
    )
    ident_in = nc.declare_dram_parameter("ident", [P, P], bf16, isOutput=False)
    dinvb = nc.declare_dram_parameter("dinvb", [P, nblk], f32, isOutput=False)
    dinvsq = nc.declare_dram_parameter("dinvsq", [P, nblk], f32, isOutput=False)
    w1 = nc.declare_dram_parameter("W1", [in_dim, hid], bf16, isOutput=False)
    w2 = nc.declare_dram_parameter("W2", [hid, out_dim], bf16, isOutput=False)
    if not bias_zero:
        b1r = nc.declare_dram_parameter("b1r", [1, hid], bf16, isOutput=False)
        b2r = nc.declare_dram_parameter("b2r", [1, out_dim], bf16, isOutput=False)
        rdinvr = nc.declare_dram_parameter(
            "rdinvr", [1, nblk * P], bf16, isOutput=False
        )
    out_ext = nc.declare_dram_parameter("out", [shard, out_dim], f32, isOutput=True)

    hloc = nc.dram_tensor("hloc", [shard, hid], bf16)
    hfull = nc.dram_tensor("hfull", [n, hid], bf16, addr_space="Shared")
    h2loc = nc.dram_tensor("h2loc", [shard, P], bf16)
    h2full = nc.dram_tensor("h2full", [n, P], bf16, addr_space="Shared")

    kin = in_dim // P  # contraction tiles for layer-1 matmul

    with tile.TileContext(nc) as tc:
        with (
            tc.tile_pool(name="const", bufs=1) as cpool,
            tc.tile_pool(name="xload", bufs=2) as xpool,
            tc.tile_pool(name="idx", bufs=8) as ipool,
            tc.tile_pool(name="gath", bufs=7) as gpool,
            tc.tile_pool(name="sind", bufs=3) as spool,
            tc.tile_pool(name="blk", bufs=8) as bpool,
            tc.tile_pool(name="psagg", bufs=1, space="PSUM") as psagg,
            tc.tile_pool(name="psh2", bufs=1, space="PSUM") as psh2,
        ):
            import contextlib

            regstack = contextlib.ExitStack()
            nreg = regstack.enter_context(nc.gpsimd.register("nreg"))
            # ---- constants into SBUF
            ident_sb = cpool.tile([P, P], bf16, tag="ident")
            nc.sync.dma_start(out=ident_sb[:], in_=ident_in[:])
            w1_t = [
                cpool.tile([P, hid], bf16, tag=f"w1_{k}", name=f"w1t{k}")
                for k in range(kin)
            ]
            for k in range(kin):
                nc.sync.dma_start(out=w1_t[k][:], in_=w1[k * P : (k + 1) * P, :])
            w2_sb = cpool.tile([hid, out_dim], bf16, tag="w2")
            nc.sync.dma_start(out=w2_sb[:], in_=w2[:])
            dinvb_sb = cpool.tile([P, nblk], f32, tag="dinvb")
            nc.sync.dma_start(out=dinvb_sb[:], in_=dinvb[:])
            dinvsq_sb = cpool.tile([P, nblk], f32, tag="dinvsq")
            nc.sync.dma_start(out=dinvsq_sb[:], in_=dinvsq[:])
            dstloc_sb = cpool.tile([P, sch.ninst], bf16, tag="dstloc")
            nc.sync.dma_start(out=dstloc_sb[:], in_=dstloc[:])
            gcnt_sb = cpool.tile([1, sch.ngath], mybir.dt.int32, tag="gcnt")
            nc.sync.dma_start(out=gcnt_sb[:], in_=gcnt_in[:])
            iotar_sb = cpool.tile([P, IOTAR_K * P], bf16, tag="iotar")
            nc.sync.dma_start(out=iotar_sb[:], in_=iotar_in[:])
            if not bias_zero:
                b1_sb = cpool.tile([1, hid], bf16, tag="b1r")
                nc.sync.dma_start(out=b1_sb[:], in_=b1r[:])
                b2_sb = cpool.tile([1, out_dim], bf16, tag="b2r")
                nc.sync.dma_start(out=b2_sb[:], in_=b2r[:])
                rdinv_sb = cpool.tile([1, nblk * P], bf16, tag="rdinvr")
                nc.sync.dma_start(out=rdinv_sb[:], in_=rdinvr[:])
            # SBUF-resident tables for self-loop adds
            hres = cpool.tile([P, nblk * hid], bf16, tag="hres")
            h2self = cpool.tile([P, nblk * out_dim], bf16, tag="h2self")

            # PSUM: matmul start=True zeroes the whole 2KB bank ("zero
            # region"), so each accumulation slot owns a full bank.
            psagg_t = [
                psagg.tile([P, 4 * P], f32, tag=f"agg{i}", name=f"aggbank{i}")
                for i in range(SB_N)
            ]

            def slot_ap(si, w):
                return psagg_t[si][:, :w]

            # ---- h~ = dinv * (x @ W1), shard-local, bf16
            XGRP = 8
            for g0 in range(0, nblk, XGRP):
                g1 = min(g0 + XGRP, nblk)
                c0, c1 = g0 * P, min(g1 * P, shard)
                xt = [
                    xpool.tile([P, XGRP * P], bf16, tag=f"xt{k}", name=f"xt{k}")
                    for k in range(kin)
                ]
                for k in range(kin):
                    nc.sync.dma_start(
                        out=xt[k][:, : c1 - c0], in_=xT[k * P : (k + 1) * P, c0:c1]
                    )
                for b in range(g0, g1):
                    m = min(P, shard - b * P)
                    hp = psagg_t[b % 4]
                    for k in range(kin):
                        nc.tensor.matmul(
                            out=hp[:m, :hid],
                            lhsT=xt[k][:, b * P - c0 : b * P - c0 + m],
                            rhs=w1_t[k][:],
                            start=(k == 0),
                            stop=(k == kin - 1),
                        )
                    nc.scalar.activation(
                        out=hres[:m, b * hid : b * hid + hid],
                        in_=hp[:m, :hid],
                        func=mybir.ActivationFunctionType.Copy,
                        scale=dinvb_sb[:m, b : b + 1],
                    )
                    nc.sync.dma_start(
                        out=hloc[b * P : b * P + m, :],
                        in_=hres[:m, b * hid : b * hid + hid],
                    )

            nc.gpsimd.collective_compute(
                "AllGather",
                mybir.AluOpType.bypass,
                ins=[hloc[:]],
                outs=[hfull[:]],
                replica_groups=[list(range(NCORES))],
            )

            gq = [0]

            def run_layer(layer):
                table = hfull if layer == 1 else h2full
                w = P if layer == 1 else out_dim
                for s in range(n_sb):
                    sb = sch.sbs[s]
                    blocks = sb["blocks"]
                    slot_of = {b: i for i, b in enumerate(blocks)}
                    # self-loop add opens each block's accumulation
                    for b in blocks:
                        m = min(P, shard - b * P)
                        si = slot_of[b]
                        if layer == 1:
                            nc.tensor.matmul(
                                out=slot_ap(si, P),
                                lhsT=hres[:m, b * hid : b * hid + hid],
                                rhs=ident_sb[:m, :],
                                start=True,
                                stop=False,
                            )
                        else:
                            nc.tensor.matmul(
                                out=slot_ap(si, out_dim),
                                lhsT=ident_sb[:m, :],
                                rhs=h2self[:m, b * out_dim : (b + 1) * out_dim],
                                start=True,
                                stop=False,
                            )
                        if not bias_zero:
                            # rank-1 bias: b1 (x) 1/dinv  /  b2/dinv
                            if layer == 1:
                                nc.tensor.matmul(
                                    out=slot_ap(si, P),
                                    lhsT=b1_sb[:, :],
                                    rhs=rdinv_sb[:, b * P : (b + 1) * P],
                                    start=False,
                                    stop=False,
                                )
                            else:
                                nc.tensor.matmul(
                                    out=slot_ap(si, out_dim),
                                    lhsT=rdinv_sb[:, b * P : (b + 1) * P],
                                    rhs=b2_sb[:, :],
                                    start=False,
                                    stop=False,
                                )
                    for g in sb["gathers"]:
                        c16, nidx, k = g["c16"], g["nidx"], g["bank"]
                        gi = g["gi"]
                        nch = nidx // P
                        ninst_g = len(g["insts"])
                        it = ipool.tile(
                            [P, GMAX_CHUNKS * 8], mybir.dt.int16, tag="it"
                        )
                        nc.sync.dma_start(
                            out=it[:, : nidx // 16],
                            in_=idxs[:, c16 : c16 + nidx // 16],
                        )
                        gt = gpool.tile([P, GMAX_CHUNKS, P], bf16, tag="gt")
                        if gq[0] < 7:
                            # first use of each pool buffer: clear stale SBUF
                            # so trimmed (unwritten) slots stay finite
                            nc.vector.memset(gt[:], 0.0)
                        r0 = k * sch.bank_rows
                        r1 = min(r0 + sch.bank_rows, n)
                        nc.gpsimd.reg_load(nreg, gcnt_sb[:1, gi : gi + 1])
                        nc.gpsimd.dma_gather(
                            out_ap=gt[:, :nch, :],
                            in_ap=table[r0:r1, :],
                            idxs_ap=it[:, : nidx // 16],
                            num_idxs=nidx,
                            num_idxs_reg=nreg,
                            elem_size=P,
                            single_packet=False,
                            queue_num=gq[0] % NQUEUES,
                        )
                        gq[0] += 1
                        # batched indicator build for all instances
                        sbig = spool.tile([P, IOTAR_K, P], bf16, tag="sind")
                        nc.vector.tensor_tensor(
                            out=sbig[:, :ninst_g, :],
                            in0=iotar_sb[:, : ninst_g * P].rearrange(
                                "p (k f) -> p k f", k=ninst_g
                            ),
                            in1=dstloc_sb[
                                :, g["i0"] : g["i0"] + ninst_g
                            ].to_broadcast([P, ninst_g, P]),
                            op=mybir.AluOpType.is_equal,
                        )
                        for ii, (j, b) in enumerate(g["insts"]):
                            si = slot_of[b]
                            sp = g["stops"][ii]
                            if layer == 1:
                                nc.tensor.matmul(
                                    out=slot_ap(si, P),
                                    lhsT=gt[:, j, :],
                                    rhs=sbig[:, ii, :],
                                    start=False,
                                    stop=sp,
                                )
                            else:
                                nc.tensor.matmul(
                                    out=slot_ap(si, out_dim),
                                    lhsT=sbig[:, ii, :],
                                    rhs=gt[:, j, :out_dim],
                                    start=False,
                                    stop=sp,
                                )
                    # ---- block epilogues (Scalar + TensorE only)
                    for b in blocks:
                        m = min(P, shard - b * P)
                        si = slot_of[b]
                        if layer == 1:
                            o1 = bpool.tile([P, P], bf16, tag="o1")
                            nc.scalar.activation(
                                out=o1[:],
                                in_=slot_ap(si, P),
                                func=mybir.ActivationFunctionType.Relu,
                            )
                            h2p = psh2.tile([P, out_dim], f32, tag="h2p")
                            nc.tensor.matmul(
                                out=h2p[:m, :],
                                lhsT=o1[:, :m],
                                rhs=w2_sb[:],
                                start=True,
                                stop=True,
                            )
                            h2s = bpool.tile([P, P], bf16, tag="h2s")
                            nc.vector.memset(h2s[:, out_dim:], 0.0)
                            nc.scalar.activation(
                                out=h2s[:m, :out_dim],
                                in_=h2p[:m, :],
                                func=mybir.ActivationFunctionType.Copy,
                                scale=dinvsq_sb[:m, b : b + 1],
                            )
                            nc.scalar.activation(
                                out=h2self[:m, b * out_dim : (b + 1) * out_dim],
                                in_=h2p[:m, :],
                                func=mybir.ActivationFunctionType.Copy,
                                scale=dinvsq_sb[:m, b : b + 1],
                            )
                            nc.sync.dma_start(
                                out=h2loc[b * P : b * P + m, :], in_=h2s[:m, :]
                            )
                        else:
                            ob = bpool.tile([P, out_dim], f32, tag="ob")
                            nc.scalar.activation(
                                out=ob[:m, :],
                                in_=slot_ap(si, out_dim)[:m, :],
                                func=mybir.ActivationFunctionType.Copy,
                                scale=dinvb_sb[:m, b : b + 1],
                            )
                            nc.sync.dma_start(
                                out=out_ext[b * P : b * P + m, :], in_=ob[:m, :]
                            )

            run_layer(1)
            nc.gpsimd.collective_compute(
                "AllGather",
                mybir.AluOpType.bypass,
                ins=[h2loc[:]],
                outs=[h2full[:]],
                replica_groups=[list(range(NCORES))],
            )
            run_layer(2)
            regstack.close()

    nc.compile()
    return nc


# ---------------------------------------------------------------- kernel ---
def _make_in_maps(sch, x, W1, b1v, W2, b2v, bias_zero):
    hid = W1.shape[1]
    out_dim = W2.shape[1]
    shard = sch.shard
    nblk = sch.nblk
    bf = ml_dtypes.bfloat16
    in_maps = []
    w1b = W1.astype(bf)
    w2b = W2.astype(bf)
    iotar = np.tile(np.arange(P, dtype=np.float32), (P, IOTAR_K)).astype(bf)
    ident = np.eye(P, dtype=np.float32).astype(bf)
    for c in range(NCORES):
        xs = np.ascontiguousarray(x[c * shard : (c + 1) * shard].astype(bf).T)
        dv = sch.dinv[c * shard : (c + 1) * shard].astype(np.float64)
        full = np.zeros(nblk * P, np.float64)
        full[:shard] = dv
        cols = np.ascontiguousarray(full.reshape(nblk, P).T)
        m = {
            "xT": xs,
            "idxs": sch.idx_stream[c],
            "dstloc": sch.dstloc_s[c],
            "gcnt": sch.gcnt[c].reshape(1, -1),
            "iotar": iotar,
            "ident": ident,
            "dinvb": cols.astype(np.float32),
            "dinvsq": (cols**2).astype(np.float32),
            "W1": w1b,
            "W2": w2b,
        }
        if not bias_zero:
            rd = np.zeros(nblk * P, np.float64)
            rd[:shard] = 1.0 / dv
            m["b1r"] = b1v.reshape(1, hid).astype(bf)
            m["b2r"] = b2v.reshape(1, out_dim).astype(bf)
            m["rdinvr"] = rd.reshape(1, nblk * P).astype(bf)
        in_maps.append(m)
    return in_maps


def _get_compiled(n, e, edge_index, in_dim, hid, out_dim, bias_zero):
    key = ("nc", n, e, bias_zero)
    if key not in _CACHE:
        sch = _preprocess(n, edge_index)
        _CACHE[("sched", n, e)] = sch
        _CACHE[key] = _build(sch, in_dim, hid, out_dim, bias_zero)
    return _CACHE[("sched", n, e)], _CACHE[key]


def kernel(x, edge_index, W1, b1, W2, b2):
    _install_compat()
    from concourse.bass_utils import run_bass_kernel_spmd

    x = np.asarray(x)
    edge_index = np.asarray(edge_index)
    W1 = np.asarray(W1, np.float32)
    b1v = np.asarray(b1, np.float32)
    W2 = np.asarray(W2, np.float32)
    b2v = np.asarray(b2, np.float32)
    n, in_dim = x.shape
    hid = W1.shape[1]
    out_dim = W2.shape[1]
    bias_zero = bool(np.all(b1v == 0) and np.all(b2v == 0))

    sch, nc = _get_compiled(
        n, edge_index.shape[1], edge_index, in_dim, hid, out_dim, bias_zero
    )
    in_maps = _make_in_maps(sch, x, W1, b1v, W2, b2v, bias_zero)
    import os

    trace = bool(os.environ.get("GCN_TRACE"))
    res = run_bass_kernel_spmd(
        nc, in_maps, core_ids=list(range(NCORES)), trace=trace
    )
    global LAST_EXEC_NS
    LAST_EXEC_NS = res.exec_time_ns
    return np.concatenate([res.results[c]["out"] for c in range(NCORES)], axis=0)


LAST_EXEC_NS = None


# revision 53
# speedup vs baseline: 3.4556x; 1.0598x over previous
"""2-layer GCN (GCNConv x2) on 8 Trainium2 NeuronCores.

Strategy (dst-sharded, edge-partitioned by destination):
- Each core owns N/8 destination nodes and the edges pointing at them.
- h~ = dinv * (x @ W1) computed per-shard, AllGathered to a full bf16 table.
- Per-edge messages fetched with dma_gather (4 SWDGE queues round-robin);
  scatter-add done as one-hot-indicator matmuls accumulating in PSUM.
- Q7 descriptor generation is the critical resource, so the schedule
  minimizes gathered slots: no self-loop edges (self term is an
  identity-matmul add of SBUF-resident rows), per-(superblock, bank)
  chunking with per-block max-core slot packing, trailing -1 indices so
  each core only generates descriptors for its real edge count.
- Epilogues run on Scalar/TensorE only (relu(dinv*x) = dinv*relu(x));
  DVE just builds indicator matrices (batched is_equal per gather).
"""
import sys
import types

import numpy as np
import ml_dtypes

P = 128
NCORES = 8
GMAX_CHUNKS = 40  # max chunks (128 idxs each) per dma_gather
SB_N = 7  # dst blocks per super-block = PSUM accumulation banks
NQUEUES = 4
IOTAR_K = 48  # max instances per batched is_eq

_CACHE = {}


# ---------------------------------------------------------------- compat ---
def _install_compat():
    """Patches for this axon/walrus stack (drain waits, per-inst wait caps,
    NTFF shim). Idempotent."""
    if _CACHE.get("compat"):
        return
    import concourse.tile as tile
    import concourse.mybir as mybir

    _ev = [0]

    def _split_inst_waits(ordered):
        for _bb, insts in ordered.items():
            out = []
            for inst in insts:
                si = getattr(inst, "sync_info", None)
                if si is not None and si.on_wait is not None and len(si.on_wait) > 1:
                    waits = list(si.on_wait)
                    excess, keep = waits[:-1], waits[-1:]
                    si.on_wait.clear()
                    for sw in keep:
                        si.on_wait.append(sw)
                    for i in range(0, len(excess), 2):
                        _ev[0] += 1
                        ev = mybir.InstEventSemaphore(
                            name=f"evsplit-{_ev[0]}", ins=[], outs=[]
                        )
                        ev.engine = inst.engine
                        ev.sync_info = mybir.SyncInfo(
                            on_wait=excess[i : i + 2], on_update=[]
                        )
                        out.append(ev)
                out.append(inst)
            insts[:] = out

    orig_lower = tile.TileContext._lower_ordered_insts

    def patched_lower(self, ordered):
        _split_inst_waits(ordered)
        return orig_lower(self, ordered)

    def patched_drain(self, tick_clock, wait_clock):
        sems_alloc = list(self.sems.allocated().values())
        carrier = self.nc.sync.wait_ge(sems_alloc[0], 0)
        wait_clock.add_sem_waits(
            carrier.ins, tile.ScopedClock({None: tick_clock.global_clock})
        )
        waits = list(carrier.ins.sync_info.on_wait)
        carrier.ins.sync_info.on_wait.clear()
        for sw in waits[:2]:
            carrier.ins.sync_info.on_wait.append(sw)
        for i in range(2, len(waits), 2):
            c = self.nc.sync.wait_ge(sems_alloc[0], 0)
            c.ins.sync_info.on_wait.clear()
            for sw in waits[i : i + 2]:
                c.ins.sync_info.on_wait.append(sw)
        self.nc.sync.drain(fusable=False)
        self.nc.all_engine_barrier()
        popped = self.nc._tile_sem_poison_stack.pop()
        assert popped is self._sem_poison
        self.nc.clear_and_free_semaphores(sems_alloc)
        self.nc.all_engine_barrier()

    tile.TileContext._lower_ordered_insts = patched_lower
    tile.TileContext._drain_and_barrier = patched_drain

    # NTFF profile hook shim (missing antenv.axon_hooks in this image)
    _hook = {}
    mod = types.ModuleType("antenv.axon_hooks")
    mod.set_axon_ntff_profile_hook = lambda h: _hook.update(hook=h)
    mod.get_axon_ntff_profile_hook = lambda: _hook.get("hook")
    sys.modules["antenv.axon_hooks"] = mod
    try:
        import antenv

        antenv.axon_hooks = mod
        from trn_agent_boot.trn_boot import _ntff_profile_via_ctypes

        mod.set_axon_ntff_profile_hook(
            _ntff_profile_via_ctypes("/opt/axon/libaxon_pjrt.so")
        )
    except Exception:
        pass
    _CACHE["compat"] = True


# ---------------------------------------------------------- preprocessing ---
class Schedule:
    pass


def _round_up(v, m):
    return (v + m - 1) // m * m


def _preprocess(n, edge_index):
    """Build the uniform cross-core schedule + per-core data streams.

    Layout: for each superblock s (SB_N dst blocks), for each src bank k,
    a segment holding each block's edges padded to the max count over
    cores, chunked into 128-slot units.  Chunks may span block boundaries;
    each (chunk, block) pair is one matmul instance with its own
    indicator column stream.
    """
    shard = n // NCORES
    nblk = (shard + P - 1) // P
    nbanks = 4
    qs = shard // nbanks
    bank_rows = (n + nbanks - 1) // nbanks
    assert bank_rows <= 32767
    n_sb = (nblk + SB_N - 1) // SB_N

    src = edge_index[0].astype(np.int64)
    dst = edge_index[1].astype(np.int64)
    e = src.shape[0]
    deg = np.bincount(dst, minlength=n).astype(np.float64) + 1.0
    dinv = (1.0 / np.sqrt(deg)).astype(np.float32)

    core = dst // shard
    dl = dst - core * shard
    blk = dl // P
    dstloc = (dl % P).astype(np.int32)
    bank = src // bank_rows
    bidx = (src % bank_rows).astype(np.int32)

    # per (core, blk, bank) counts -> per-block max over cores
    cnt = np.bincount(
        (core * nblk + blk) * nbanks + bank, minlength=NCORES * nblk * nbanks
    ).reshape(NCORES, nblk, nbanks)
    mx = cnt.max(axis=0)  # [nblk, nbanks]

    # ---- compile-time program ----------------------------------------
    # gathers: (c16, nidx, bank, sb, i0, ninst) ; instances: (g, j, b, stop)
    sbs = []
    tot_slots = 0
    ninst_tot = 0
    gathers_all = []
    # chunk slot ranges per (sb, bank): block boundaries inside the segment
    seg_info = {}  # (s, k) -> (slot_base, [(b, off, mxcnt)...], nch_padded)
    for s in range(n_sb):
        blocks = list(range(s * SB_N, min((s + 1) * SB_N, nblk)))
        sb = {"blocks": blocks, "gathers": [], "instances": []}
        # last accumulate instance per block for stop flags
        for k in range(nbanks):
            seg_blocks = []
            off = 0
            for b in blocks:
                seg_blocks.append((b, off, int(mx[b, k])))
                off += int(mx[b, k])
            nseg = off
            nch_real = (nseg + P - 1) // P
            # split into gathers of <= GMAX chunks, pad each to mult of 4
            c0 = 0
            gath_list = []
            while c0 < nch_real:
                gn_real = min(GMAX_CHUNKS, nch_real - c0)
                gath_list.append((c0, gn_real, gn_real))
                c0 += gn_real
            seg_info[(s, k)] = (tot_slots, seg_blocks, gath_list)
            for (c0g, gn_real, gn) in gath_list:
                # instances: blocks overlapping chunks [c0g, c0g+gn_real)
                insts = []
                for j in range(gn_real):
                    lo = (c0g + j) * P
                    hi = lo + P
                    for (b, boff, bmx) in seg_blocks:
                        if boff < hi and boff + bmx > lo:
                            insts.append((j, b))
                g = {
                    "c16": tot_slots // 16,
                    "nidx": gn * P,
                    "bank": k,
                    "i0": ninst_tot,
                    "insts": insts,
                    "gi": len(gathers_all),
                }
                ninst_tot += len(insts)
                tot_slots += gn * P
                sb["gathers"].append(g)
                gathers_all.append(g)
        # stop flags: last instance of each block within this sb
        last = {}
        for g in sb["gathers"]:
            for idx, (j, b) in enumerate(g["insts"]):
                last[b] = (id(g), idx)
        for g in sb["gathers"]:
            g["stops"] = [
                last.get(b) == (id(g), idx) for idx, (j, b) in enumerate(g["insts"])
            ]
        sbs.append(sb)

    # ---- per-core streams ---------------------------------------------
    idx_stream = np.zeros((NCORES, 16, tot_slots // 16), np.int16)
    dstloc_s = np.full((NCORES, P, ninst_tot), -1.0, ml_dtypes.bfloat16)
    ngath = len(gathers_all)
    gcnt = np.zeros((NCORES, ngath), np.int32)

    order = np.lexsort((bidx, blk, bank, core))
    so_core = core[order]
    so_blk = blk[order]
    so_bank = bank[order]
    so_bidx = bidx[order]
    so_dstloc = dstloc[order]
    ptr = np.searchsorted(so_core, np.arange(NCORES + 1))

    for c in range(NCORES):
        lo, hi = ptr[c], ptr[c + 1]
        cblk = so_blk[lo:hi]
        cbank = so_bank[lo:hi]
        cbidx = so_bidx[lo:hi]
        cdl = so_dstloc[lo:hi]
        # run starts per (bank, blk): sorted by (bank, blk)
        key = cbank * nblk + cblk
        idx_flat = np.full(tot_slots, -1, np.int32)
        dl_flat = np.full(tot_slots, -1, np.int32)
        blk_flat = np.full(tot_slots, -2, np.int32)
        real = np.zeros(tot_slots, bool)
        for s in range(n_sb):
            for k in range(nbanks):
                slot_base, seg_blocks, gath_list = seg_info[(s, k)]
                for (b, boff, bmx) in seg_blocks:
                    want = k * nblk + b
                    p0 = np.searchsorted(key, want, "left")
                    p1 = np.searchsorted(key, want, "right")
                    cn = p1 - p0
                    assert cn <= bmx
                    sl = slot_base + boff
                    idx_flat[sl : sl + cn] = cbidx[p0:p1]
                    dl_flat[sl : sl + cn] = cdl[p0:p1]
                    blk_flat[sl : sl + cn] = b
                    real[sl : sl + cn] = True
        # per-gather: pads before the last real slot gather row 0 (harmless),
        # pads after are trailing -1 (skipped by Q7 desc-gen; count goes in
        # the per-core num_idxs register).
        for gi, g in enumerate(gathers_all):
            c16, nidx = g["c16"], g["nidx"]
            w0 = c16 * 16
            rw = real[w0 : w0 + nidx]
            nz = np.nonzero(rw)[0]
            last = int(nz[-1]) + 1 if len(nz) else 0
            gcnt[c, gi] = last
            w = idx_flat[w0 : w0 + nidx]
            w[:last][~rw[:last]] = 0
            w[last:] = -1
            idx_stream[c][:, c16 : c16 + nidx // 16] = (
                w.reshape(nidx // 16, 16).T.astype(np.int16)
            )
            # instance dstloc columns
            for ii, (j, b) in enumerate(g["insts"]):
                cslots = slice(c16 * 16 + j * P, c16 * 16 + (j + 1) * P)
                m = blk_flat[cslots] == b
                col = np.where(m, dl_flat[cslots], -1).astype(np.float32)
                dstloc_s[c][:, g["i0"] + ii] = col.astype(ml_dtypes.bfloat16)

    sch = Schedule()
    sch.n, sch.e, sch.shard, sch.nblk, sch.nbanks = n, e, shard, nblk, nbanks
    sch.bank_rows, sch.n_sb, sch.qs = bank_rows, n_sb, qs
    sch.tot_slots = tot_slots
    sch.ninst = ninst_tot
    sch.ngath = ngath
    sch.gcnt = gcnt
    sch.sbs = sbs
    sch.dinv = dinv
    sch.idx_stream = np.tile(idx_stream, (1, 8, 1))  # replicate to 128 parts
    sch.dstloc_s = dstloc_s
    return sch


# ----------------------------------------------------------------- build ---
def _build(sch, in_dim, hid, out_dim, bias_zero):
    import concourse.mybir as mybir
    import concourse.tile as tile
    from concourse import bacc

    bf16 = mybir.dt.bfloat16
    f32 = mybir.dt.float32
    shard, nblk, nbanks = sch.shard, sch.nblk, sch.nbanks
    n_sb = sch.n_sb
    n = sch.n

    nc = bacc.Bacc(num_swdge_queues=NQUEUES, dynamic_dma_scratch_size=32768)

    xT = nc.declare_dram_parameter("xT", [in_dim, shard], bf16, isOutput=False)
    idxs = nc.declare_dram_parameter(
        "idxs", [P, sch.tot_slots // 16], mybir.dt.int16, isOutput=False
    )
    dstloc = nc.declare_dram_parameter("dstloc", [P, sch.ninst], bf16, isOutput=False)
    gcnt_in = nc.declare_dram_parameter(
        "gcnt", [1, sch.ngath], mybir.dt.int32, isOutput=False
    )
    iotar_in = nc.declare_dram_parameter(
        "iotar", [P, IOTAR_K * P], bf16, isOutput=False
    )
    ident_in = nc.declare_dram_parameter("ident", [P, P], bf16, isOutput=False)
    dinvb = nc.declare_dram_parameter("dinvb", [P, nblk], f32, isOutput=False)
    dinvsq = nc.declare_dram_parameter("dinvsq", [P, nblk], f32, isOutput=False)
    w1 = nc.declare_dram_parameter("W1", [in_dim, hid], bf16, isOutput=False)
    w2 = nc.declare_dram_parameter("W2", [hid, out_dim], bf16, isOutput=False)
    if not bias_zero:
        b1r = nc.declare_dram_parameter("b1r", [1, hid], bf16, isOutput=False)
        b2r = nc.declare_dram_parameter("b2r", [1, out_dim], bf16, isOutput=False)
        rdinvr = nc.declare_dram_parameter(
            "rdinvr", [1, nblk * P], bf16, isOutput=False
        )
    out_ext = nc.declare_dram_parameter("out", [shard, out_dim], f32, isOutput=True)

    hloc = nc.dram_tensor("hloc", [shard, hid], bf16)
    hfull = nc.dram_tensor("hfull", [n, hid], bf16, addr_space="Shared")
    h2loc = nc.dram_tensor("h2loc", [shard, P], bf16)
    h2full = nc.dram_tensor("h2full", [n, P], bf16, addr_space="Shared")

    kin = in_dim // P  # contraction tiles for layer-1 matmul

    with tile.TileContext(nc) as tc:
        with (
            tc.tile_pool(name="const", bufs=1) as cpool,
            tc.tile_pool(name="xload", bufs=2) as xpool,
            tc.tile_pool(name="idx", bufs=8) as ipool,
            tc.tile_pool(name="gath", bufs=7) as gpool,
            tc.tile_pool(name="sind", bufs=3) as spool,
            tc.tile_pool(name="blk", bufs=12) as bpool,
            tc.tile_pool(name="psagg", bufs=1, space="PSUM") as psagg,
            tc.tile_pool(name="psh2", bufs=1, space="PSUM") as psh2,
        ):
            import contextlib

            regstack = contextlib.ExitStack()
            nreg = regstack.enter_context(nc.gpsimd.register("nreg"))
            # ---- constants into SBUF
            ident_sb = cpool.tile([P, P], bf16, tag="ident")
            nc.sync.dma_start(out=ident_sb[:], in_=ident_in[:])
            w1_t = [
                cpool.tile([P, hid], bf16, tag=f"w1_{k}", name=f"w1t{k}")
                for k in range(kin)
            ]
            for k in range(kin):
                nc.sync.dma_start(out=w1_t[k][:], in_=w1[k * P : (k + 1) * P, :])
            w2_sb = cpool.tile([hid, out_dim], bf16, tag="w2")
            nc.sync.dma_start(out=w2_sb[:], in_=w2[:])
            dinvb_sb = cpool.tile([P, nblk], f32, tag="dinvb")
            nc.sync.dma_start(out=dinvb_sb[:], in_=dinvb[:])
            dinvsq_sb = cpool.tile([P, nblk], f32, tag="dinvsq")
            nc.sync.dma_start(out=dinvsq_sb[:], in_=dinvsq[:])
            dstloc_sb = cpool.tile([P, sch.ninst], bf16, tag="dstloc")
            nc.sync.dma_start(out=dstloc_sb[:], in_=dstloc[:])
            gcnt_sb = cpool.tile([1, sch.ngath], mybir.dt.int32, tag="gcnt")
            nc.sync.dma_start(out=gcnt_sb[:], in_=gcnt_in[:])
            iotar_sb = cpool.tile([P, IOTAR_K * P], bf16, tag="iotar")
            nc.sync.dma_start(out=iotar_sb[:], in_=iotar_in[:])
            if not bias_zero:
                b1_sb = cpool.tile([1, hid], bf16, tag="b1r")
                nc.sync.dma_start(out=b1_sb[:], in_=b1r[:])
                b2_sb = cpool.tile([1, out_dim], bf16, tag="b2r")
                nc.sync.dma_start(out=b2_sb[:], in_=b2r[:])
                rdinv_sb = cpool.tile([1, nblk * P], bf16, tag="rdinvr")
                nc.sync.dma_start(out=rdinv_sb[:], in_=rdinvr[:])
            # SBUF-resident tables for self-loop adds
            hres = cpool.tile([P, nblk * hid], bf16, tag="hres")
            h2self = cpool.tile([P, nblk * out_dim], bf16, tag="h2self")

            # PSUM: matmul start=True zeroes the whole 2KB bank ("zero
            # region"), so each accumulation slot owns a full bank.
            psagg_t = [
                psagg.tile([P, 4 * P], f32, tag=f"agg{i}", name=f"aggbank{i}")
                for i in range(SB_N)
            ]

            def slot_ap(si, w):
                return psagg_t[si][:, :w]

            # ---- h~ = dinv * (x @ W1), shard-local, bf16
            XGRP = 8
            for g0 in range(0, nblk, XGRP):
                g1 = min(g0 + XGRP, nblk)
                c0, c1 = g0 * P, min(g1 * P, shard)
                xt = [
                    xpool.tile([P, XGRP * P], bf16, tag=f"xt{k}", name=f"xt{k}")
                    for k in range(kin)
                ]
                for k in range(kin):
                    nc.sync.dma_start(
                        out=xt[k][:, : c1 - c0], in_=xT[k * P : (k + 1) * P, c0:c1]
                    )
                for b in range(g0, g1):
                    m = min(P, shard - b * P)
                    hp = psagg_t[b % 4]
                    for k in range(kin):
                        nc.tensor.matmul(
                            out=hp[:m, :hid],
                            lhsT=xt[k][:, b * P - c0 : b * P - c0 + m],
                            rhs=w1_t[k][:],
                            start=(k == 0),
                            stop=(k == kin - 1),
                        )
                    nc.scalar.activation(
                        out=hres[:m, b * hid : b * hid + hid],
                        in_=hp[:m, :hid],
                        func=mybir.ActivationFunctionType.Copy,
                        scale=dinvb_sb[:m, b : b + 1],
                    )
                    nc.sync.dma_start(
                        out=hloc[b * P : b * P + m, :],
                        in_=hres[:m, b * hid : b * hid + hid],
                    )

            nc.gpsimd.collective_compute(
                "AllGather",
                mybir.AluOpType.bypass,
                ins=[hloc[:]],
                outs=[hfull[:]],
                replica_groups=[list(range(NCORES))],
            )

            gq = [0]

            def run_layer(layer):
                table = hfull if layer == 1 else h2full
                w = P if layer == 1 else out_dim
                for s in range(n_sb):
                    sb = sch.sbs[s]
                    blocks = sb["blocks"]
                    slot_of = {b: i for i, b in enumerate(blocks)}
                    # self-loop add opens each block's accumulation
                    for b in blocks:
                        m = min(P, shard - b * P)
                        si = slot_of[b]
                        if layer == 1:
                            nc.tensor.matmul(
                                out=slot_ap(si, P),
                                lhsT=hres[:m, b * hid : b * hid + hid],
                                rhs=ident_sb[:m, :],
                                start=True,
                                stop=False,
                            )
                        else:
                            nc.tensor.matmul(
                                out=slot_ap(si, out_dim),
                                lhsT=ident_sb[:m, :],
                                rhs=h2self[:m, b * out_dim : (b + 1) * out_dim],
                                start=True,
                                stop=False,
                            )
                        if not bias_zero:
                            # rank-1 bias: b1 (x) 1/dinv  /  b2/dinv
                            if layer == 1:
                                nc.tensor.matmul(
                                    out=slot_ap(si, P),
                                    lhsT=b1_sb[:, :],
                                    rhs=rdinv_sb[:, b * P : (b + 1) * P],
                                    start=False,
                                    stop=False,
                                )
                            else:
                                nc.tensor.matmul(
                                    out=slot_ap(si, out_dim),
                                    lhsT=rdinv_sb[:, b * P : (b + 1) * P],
                                    rhs=b2_sb[:, :],
                                    start=False,
                                    stop=False,
                                )
                    for g in sb["gathers"]:
                        c16, nidx, k = g["c16"], g["nidx"], g["bank"]
                        gi = g["gi"]
                        nch = nidx // P
                        ninst_g = len(g["insts"])
                        it = ipool.tile(
                            [P, GMAX_CHUNKS * 8], mybir.dt.int16, tag="it"
                        )
                        nc.sync.dma_start(
                            out=it[:, : nidx // 16],
                            in_=idxs[:, c16 : c16 + nidx // 16],
                        )
                        gt = gpool.tile([P, GMAX_CHUNKS, P], bf16, tag="gt")
                        if gq[0] < 7:
                            # first use of each pool buffer: clear stale SBUF
                            # so trimmed (unwritten) slots stay finite
                            nc.vector.memset(gt[:], 0.0)
                        r0 = k * sch.bank_rows
                        r1 = min(r0 + sch.bank_rows, n)
                        nc.gpsimd.reg_load(nreg, gcnt_sb[:1, gi : gi + 1])
                        nc.gpsimd.dma_gather(
                            out_ap=gt[:, :nch, :],
                            in_ap=table[r0:r1, :],
                            idxs_ap=it[:, : nidx // 16],
                            num_idxs=nidx,
                            num_idxs_reg=nreg,
                            elem_size=P,
                            single_packet=False,
                            queue_num=gq[0] % NQUEUES,
                        )
                        gq[0] += 1
                        # batched indicator build for all instances
                        sbig = spool.tile([P, IOTAR_K, P], bf16, tag="sind")
                        nc.vector.tensor_tensor(
                            out=sbig[:, :ninst_g, :],
                            in0=iotar_sb[:, : ninst_g * P].rearrange(
                                "p (k f) -> p k f", k=ninst_g
                            ),
                            in1=dstloc_sb[
                                :, g["i0"] : g["i0"] + ninst_g
                            ].to_broadcast([P, ninst_g, P]),
                            op=mybir.AluOpType.is_equal,
                        )
                        for ii, (j, b) in enumerate(g["insts"]):
                            si = slot_of[b]
                            sp = g["stops"][ii]
                            if layer == 1:
                                nc.tensor.matmul(
                                    out=slot_ap(si, P),
                                    lhsT=gt[:, j, :],
                                    rhs=sbig[:, ii, :],
                                    start=False,
                                    stop=sp,
                                )
                            else:
                                nc.tensor.matmul(
                                    out=slot_ap(si, out_dim),
                                    lhsT=sbig[:, ii, :],
                                    rhs=gt[:, j, :out_dim],
                                    start=False,
                                    stop=sp,
                                )
                    # ---- block epilogues (Scalar + TensorE only)
                    for b in blocks:
                        m = min(P, shard - b * P)
                        si = slot_of[b]
                        if layer == 1:
                            o1 = bpool.tile([P, P], bf16, tag="o1")
                            nc.scalar.activation(
                                out=o1[:],
                                in_=slot_ap(si, P),
                                func=mybir.ActivationFunctionType.Relu,
                            )
                            h2p = psh2.tile([P, out_dim], f32, tag="h2p")
                            nc.tensor.matmul(
                                out=h2p[:m, :],
                                lhsT=o1[:, :m],
                                rhs=w2_sb[:],
                                start=True,
                                stop=True,
                            )
                            h2s = bpool.tile([P, P], bf16, tag="h2s")
                            nc.vector.memset(h2s[:, out_dim:], 0.0)
                            nc.scalar.activation(
                                out=h2s[:m, :out_dim],
                                in_=h2p[:m, :],
                                func=mybir.ActivationFunctionType.Copy,
                                scale=dinvsq_sb[:m, b : b + 1],
                            )
                            nc.scalar.activation(
                                out=h2self[:m, b * out_dim : (b + 1) * out_dim],
                                in_=h2p[:m, :],
                                func=mybir.ActivationFunctionType.Copy,
                                scale=dinvsq_sb[:m, b : b + 1],
                            )
                            nc.sync.dma_start(
                                out=h2loc[b * P : b * P + m, :], in_=h2s[:m, :]
                            )
                        else:
                            ob = bpool.tile([P, out_dim], f32, tag="ob")
                            nc.scalar.activation(
                                out=ob[:m, :],
                                in_=slot_ap(si, out_dim)[:m, :],
                                func=mybir.ActivationFunctionType.Copy,
                                scale=dinvb_sb[:m, b : b + 1],
                            )
                            nc.sync.dma_start(
                                out=out_ext[b * P : b * P + m, :], in_=ob[:m, :]
                            )

            run_layer(1)
            nc.gpsimd.collective_compute(
                "AllGather",
                mybir.AluOpType.bypass,
                ins=[h2loc[:]],
                outs=[h2full[:]],
                replica_groups=[list(range(NCORES))],
            )
            run_layer(2)
            regstack.close()

    nc.compile()
    return nc


# ---------------------------------------------------------------- kernel ---
def _make_in_maps(sch, x, W1, b1v, W2, b2v, bias_zero):
    hid = W1.shape[1]
    out_dim = W2.shape[1]
    shard = sch.shard
    nblk = sch.nblk
    bf = ml_dtypes.bfloat16
    in_maps = []
    w1b = W1.astype(bf)
    w2b = W2.astype(bf)
    iotar = np.tile(np.arange(P, dtype=np.float32), (P, IOTAR_K)).astype(bf)
    ident = np.eye(P, dtype=np.float32).astype(bf)
    for c in range(NCORES):
        xs = np.ascontiguousarray(x[c * shard : (c + 1) * shard].astype(bf).T)
        dv = sch.dinv[c * shard : (c + 1) * shard].astype(np.float64)
        full = np.zeros(nblk * P, np.float64)
        full[:shard] = dv
        cols = np.ascontiguousarray(full.reshape(nblk, P).T)
        m = {
            "xT": xs,
            "idxs": sch.idx_stream[c],
            "dstloc": sch.dstloc_s[c],
            "gcnt": sch.gcnt[c].reshape(1, -1),
            "iotar": iotar,
            "ident": ident,
            "dinvb": cols.astype(np.float32),
            "dinvsq": (cols**2).astype(np.float32),
            "W1": w1b,
            "W2": w2b,
        }
        if not bias_zero:
            rd = np.zeros(nblk * P, np.float64)
            rd[:shard] = 1.0 / dv
            m["b1r"] = b1v.reshape(1, hid).astype(bf)
            m["b2r"] = b2v.reshape(1, out_dim).astype(bf)
            m["rdinvr"] = rd.reshape(1, nblk * P).astype(bf)
        in_maps.append(m)
    return in_maps


def _get_compiled(n, e, edge_index, in_dim, hid, out_dim, bias_zero):
    key = ("nc", n, e, bias_zero)
    if key not in _CACHE:
        sch = _preprocess(n, edge_index)
        _CACHE[("sched", n, e)] = sch
        _CACHE[key] = _build(sch, in_dim, hid, out_dim, bias_zero)
    return _CACHE[("sched", n, e)], _CACHE[key]


def kernel(x, edge_index, W1, b1, W2, b2):
    _install_compat()
    from concourse.bass_utils import run_bass_kernel_spmd

    x = np.asarray(x)
    edge_index = np.asarray(edge_index)
    W1 = np.asarray(W1, np.float32)
    b1v = np.asarray(b1, np.float32)
    W2 = np.asarray(W2, np.float32)
    b2v = np.asarray(b2, np.float32)
    n, in_dim = x.shape
    hid = W1.shape[1]
    out_dim = W2.shape[1]
    bias_zero = bool(np.all(b1v == 0) and np.all(b2v == 0))

    sch, nc = _get_compiled(
        n, edge_index.shape[1], edge_index, in_dim, hid, out_dim, bias_zero
    )
    in_maps = _make_in_maps(sch, x, W1, b1v, W2, b2v, bias_zero)
    import os

    trace = bool(os.environ.get("GCN_TRACE"))
    res = run_bass_kernel_spmd(
        nc, in_maps, core_ids=list(range(NCORES)), trace=trace
    )
    global LAST_EXEC_NS
    LAST_EXEC_NS = res.exec_time_ns
    return np.concatenate([res.results[c]["out"] for c in range(NCORES)], axis=0)


LAST_EXEC_NS = None


# revision 56
# speedup vs baseline: 3.4593x; 1.0011x over previous
"""2-layer GCN (GCNConv x2) on 8 Trainium2 NeuronCores.

Strategy (dst-sharded, edge-partitioned by destination):
- Each core owns N/8 destination nodes and the edges pointing at them.
- h~ = dinv * (x @ W1) computed per-shard, AllGathered to a full bf16 table.
- Per-edge messages fetched with dma_gather (4 SWDGE queues round-robin);
  scatter-add done as one-hot-indicator matmuls accumulating in PSUM.
- Q7 descriptor generation is the critical resource, so the schedule
  minimizes gathered slots: no self-loop edges (self term is an
  identity-matmul add of SBUF-resident rows), per-(superblock, bank)
  chunking with per-block max-core slot packing, trailing -1 indices so
  each core only generates descriptors for its real edge count.
- Epilogues run on Scalar/TensorE only (relu(dinv*x) = dinv*relu(x));
  DVE just builds indicator matrices (batched is_equal per gather).
"""
import sys
import types

import numpy as np
import ml_dtypes

P = 128
NCORES = 8
GMAX_CHUNKS = 40  # max chunks (128 idxs each) per dma_gather
SB_N = 7  # dst blocks per super-block = PSUM accumulation banks
NQUEUES = 4
IOTAR_K = 48  # max instances per batched is_eq

_CACHE = {}


# ---------------------------------------------------------------- compat ---
def _install_compat():
    """Patches for this axon/walrus stack (drain waits, per-inst wait caps,
    NTFF shim). Idempotent."""
    if _CACHE.get("compat"):
        return
    import concourse.tile as tile
    import concourse.mybir as mybir

    _ev = [0]

    def _split_inst_waits(ordered):
        for _bb, insts in ordered.items():
            out = []
            for inst in insts:
                si = getattr(inst, "sync_info", None)
                if si is not None and si.on_wait is not None and len(si.on_wait) > 1:
                    waits = list(si.on_wait)
                    excess, keep = waits[:-1], waits[-1:]
                    si.on_wait.clear()
                    for sw in keep:
                        si.on_wait.append(sw)
                    for i in range(0, len(excess), 2):
                        _ev[0] += 1
                        ev = mybir.InstEventSemaphore(
                            name=f"evsplit-{_ev[0]}", ins=[], outs=[]
                        )
                        ev.engine = inst.engine
                        ev.sync_info = mybir.SyncInfo(
                            on_wait=excess[i : i + 2], on_update=[]
                        )
                        out.append(ev)
                out.append(inst)
            insts[:] = out

    orig_lower = tile.TileContext._lower_ordered_insts

    def patched_lower(self, ordered):
        _split_inst_waits(ordered)
        return orig_lower(self, ordered)

    def patched_drain(self, tick_clock, wait_clock):
        sems_alloc = list(self.sems.allocated().values())
        carrier = self.nc.sync.wait_ge(sems_alloc[0], 0)
        wait_clock.add_sem_waits(
            carrier.ins, tile.ScopedClock({None: tick_clock.global_clock})
        )
        waits = list(carrier.ins.sync_info.on_wait)
        carrier.ins.sync_info.on_wait.clear()
        for sw in waits[:2]:
            carrier.ins.sync_info.on_wait.append(sw)
        for i in range(2, len(waits), 2):
            c = self.nc.sync.wait_ge(sems_alloc[0], 0)
            c.ins.sync_info.on_wait.clear()
            for sw in waits[i : i + 2]:
                c.ins.sync_info.on_wait.append(sw)
        self.nc.sync.drain(fusable=False)
        self.nc.all_engine_barrier()
        popped = self.nc._tile_sem_poison_stack.pop()
        assert popped is self._sem_poison
        self.nc.clear_and_free_semaphores(sems_alloc)
        self.nc.all_engine_barrier()

    tile.TileContext._lower_ordered_insts = patched_lower
    tile.TileContext._drain_and_barrier = patched_drain

    # NTFF profile hook shim (missing antenv.axon_hooks in this image)
    _hook = {}
    mod = types.ModuleType("antenv.axon_hooks")
    mod.set_axon_ntff_profile_hook = lambda h: _hook.update(hook=h)
    mod.get_axon_ntff_profile_hook = lambda: _hook.get("hook")
    sys.modules["antenv.axon_hooks"] = mod
    try:
        import antenv

        antenv.axon_hooks = mod
        from trn_agent_boot.trn_boot import _ntff_profile_via_ctypes

        mod.set_axon_ntff_profile_hook(
            _ntff_profile_via_ctypes("/opt/axon/libaxon_pjrt.so")
        )
    except Exception:
        pass
    _CACHE["compat"] = True


# ---------------------------------------------------------- preprocessing ---
class Schedule:
    pass


def _round_up(v, m):
    return (v + m - 1) // m * m


def _preprocess(n, edge_index):
    """Build the uniform cross-core schedule + per-core data streams.

    Layout: for each superblock s (SB_N dst blocks), for each src bank k,
    a segment holding each block's edges padded to the max count over
    cores, chunked into 128-slot units.  Chunks may span block boundaries;
    each (chunk, block) pair is one matmul instance with its own
    indicator column stream.
    """
    shard = n // NCORES
    nblk = (shard + P - 1) // P
    nbanks = 4
    qs = shard // nbanks
    bank_rows = (n + nbanks - 1) // nbanks
    assert bank_rows <= 32767
    n_sb = (nblk + SB_N - 1) // SB_N

    src = edge_index[0].astype(np.int64)
    dst = edge_index[1].astype(np.int64)
    e = src.shape[0]
    deg = np.bincount(dst, minlength=n).astype(np.float64) + 1.0
    dinv = (1.0 / np.sqrt(deg)).astype(np.float32)

    core = dst // shard
    dl = dst - core * shard
    blk = dl // P
    dstloc = (dl % P).astype(np.int32)
    bank = src // bank_rows
    bidx = (src % bank_rows).astype(np.int32)

    # per (core, blk, bank) counts -> per-block max over cores
    cnt = np.bincount(
        (core * nblk + blk) * nbanks + bank, minlength=NCORES * nblk * nbanks
    ).reshape(NCORES, nblk, nbanks)
    mx = cnt.max(axis=0)  # [nblk, nbanks]

    # ---- compile-time program ----------------------------------------
    # gathers: (c16, nidx, bank, sb, i0, ninst) ; instances: (g, j, b, stop)
    sbs = []
    tot_slots = 0
    ninst_tot = 0
    gathers_all = []
    # chunk slot ranges per (sb, bank): block boundaries inside the segment
    seg_info = {}  # (s, k) -> (slot_base, [(b, off, mxcnt)...], nch_padded)
    for s in range(n_sb):
        blocks = list(range(s * SB_N, min((s + 1) * SB_N, nblk)))
        sb = {"blocks": blocks, "gathers": [], "instances": []}
        # last accumulate instance per block for stop flags
        for k in range(nbanks):
            seg_blocks = []
            off = 0
            for b in blocks:
                seg_blocks.append((b, off, int(mx[b, k])))
                off += int(mx[b, k])
            nseg = off
            nch_real = (nseg + P - 1) // P
            # split into gathers of <= GMAX chunks, pad each to mult of 4
            c0 = 0
            gath_list = []
            while c0 < nch_real:
                gn_real = min(GMAX_CHUNKS, nch_real - c0)
                gath_list.append((c0, gn_real, gn_real))
                c0 += gn_real
            seg_info[(s, k)] = (tot_slots, seg_blocks, gath_list)
            for (c0g, gn_real, gn) in gath_list:
                # instances: blocks overlapping chunks [c0g, c0g+gn_real)
                insts = []
                for j in range(gn_real):
                    lo = (c0g + j) * P
                    hi = lo + P
                    for (b, boff, bmx) in seg_blocks:
                        if boff < hi and boff + bmx > lo:
                            insts.append((j, b))
                g = {
                    "c16": tot_slots // 16,
                    "nidx": gn * P,
                    "bank": k,
                    "i0": ninst_tot,
                    "insts": insts,
                    "gi": len(gathers_all),
                }
                ninst_tot += len(insts)
                tot_slots += gn * P
                sb["gathers"].append(g)
                gathers_all.append(g)
        # stop flags: last instance of each block within this sb
        last = {}
        for g in sb["gathers"]:
            for idx, (j, b) in enumerate(g["insts"]):
                last[b] = (id(g), idx)
        for g in sb["gathers"]:
            g["stops"] = [
                last.get(b) == (id(g), idx) for idx, (j, b) in enumerate(g["insts"])
            ]
        sbs.append(sb)

    # ---- per-core streams ---------------------------------------------
    idx_stream = np.zeros((NCORES, 16, tot_slots // 16), np.int16)
    dstloc_s = np.full((NCORES, P, ninst_tot), -1.0, ml_dtypes.bfloat16)
    ngath = len(gathers_all)
    gcnt = np.zeros((NCORES, ngath), np.int32)

    order = np.lexsort((bidx, blk, bank, core))
    so_core = core[order]
    so_blk = blk[order]
    so_bank = bank[order]
    so_bidx = bidx[order]
    so_dstloc = dstloc[order]
    ptr = np.searchsorted(so_core, np.arange(NCORES + 1))

    for c in range(NCORES):
        lo, hi = ptr[c], ptr[c + 1]
        cblk = so_blk[lo:hi]
        cbank = so_bank[lo:hi]
        cbidx = so_bidx[lo:hi]
        cdl = so_dstloc[lo:hi]
        # run starts per (bank, blk): sorted by (bank, blk)
        key = cbank * nblk + cblk
        idx_flat = np.full(tot_slots, -1, np.int32)
        dl_flat = np.full(tot_slots, -1, np.int32)
        blk_flat = np.full(tot_slots, -2, np.int32)
        real = np.zeros(tot_slots, bool)
        for s in range(n_sb):
            for k in range(nbanks):
                slot_base, seg_blocks, gath_list = seg_info[(s, k)]
                for (b, boff, bmx) in seg_blocks:
                    want = k * nblk + b
                    p0 = np.searchsorted(key, want, "left")
                    p1 = np.searchsorted(key, want, "right")
                    cn = p1 - p0
                    assert cn <= bmx
                    sl = slot_base + boff
                    idx_flat[sl : sl + cn] = cbidx[p0:p1]
                    dl_flat[sl : sl + cn] = cdl[p0:p1]
                    blk_flat[sl : sl + cn] = b
                    real[sl : sl + cn] = True
        # per-gather: pads before the last real slot gather row 0 (harmless),
        # pads after are trailing -1 (skipped by Q7 desc-gen; count goes in
        # the per-core num_idxs register).
        for gi, g in enumerate(gathers_all):
            c16, nidx = g["c16"], g["nidx"]
            w0 = c16 * 16
            rw = real[w0 : w0 + nidx]
            nz = np.nonzero(rw)[0]
            last = int(nz[-1]) + 1 if len(nz) else 0
            gcnt[c, gi] = last
            w = idx_flat[w0 : w0 + nidx]
            w[:last][~rw[:last]] = 0
            w[last:] = -1
            idx_stream[c][:, c16 : c16 + nidx // 16] = (
                w.reshape(nidx // 16, 16).T.astype(np.int16)
            )
            # instance dstloc columns
            for ii, (j, b) in enumerate(g["insts"]):
                cslots = slice(c16 * 16 + j * P, c16 * 16 + (j + 1) * P)
                m = blk_flat[cslots] == b
                col = np.where(m, dl_flat[cslots], -1).astype(np.float32)
                dstloc_s[c][:, g["i0"] + ii] = col.astype(ml_dtypes.bfloat16)

    sch = Schedule()
    sch.n, sch.e, sch.shard, sch.nblk, sch.nbanks = n, e, shard, nblk, nbanks
    sch.bank_rows, sch.n_sb, sch.qs = bank_rows, n_sb, qs
    sch.tot_slots = tot_slots
    sch.ninst = ninst_tot
    sch.ngath = ngath
    sch.gcnt = gcnt
    sch.sbs = sbs
    sch.dinv = dinv
    sch.idx_stream = np.tile(idx_stream, (1, 8, 1))  # replicate to 128 parts
    sch.dstloc_s = dstloc_s
    return sch


# ----------------------------------------------------------------- build ---
def _build(sch, in_dim, hid, out_dim, bias_zero):
    import concourse.mybir as mybir
    import concourse.tile as tile
    from concourse import bacc

    bf16 = mybir.dt.bfloat16
    f32 = mybir.dt.float32
    shard, nblk, nbanks = sch.shard, sch.nblk, sch.nbanks
    n_sb = sch.n_sb
    n = sch.n

    nc = bacc.Bacc(num_swdge_queues=NQUEUES, dynamic_dma_scratch_size=32768)

    xT = nc.declare_dram_parameter("xT", [in_dim, shard], bf16, isOutput=False)
    idxs = nc.declare_dram_parameter(
        "idxs", [P, sch.tot_slots // 16], mybir.dt.int16, isOutput=False
    )
    dstloc = nc.declare_dram_parameter("dstloc", [P, sch.ninst], bf16, isOutput=False)
    gcnt_in = nc.declare_dram_parameter(
        "gcnt", [1, sch.ngath], mybir.dt.int32, isOutput=False
    )
    iotar_in = nc.declare_dram_parameter(
        "iotar", [P, IOTAR_K * P], bf16, isOutput=False
    )
    ident_in = nc.declare_dram_parameter("ident", [P, P], bf16, isOutput=False)
    dinvb = nc.declare_dram_parameter("dinvb", [P, nblk], f32, isOutput=False)
    dinvsq = nc.declare_dram_parameter("dinvsq", [P, nblk], f32, isOutput=False)
    w1 = nc.declare_dram_parameter("W1", [in_dim, hid], bf16, isOutput=False)
    w2 = nc.declare_dram_parameter("W2", [hid, out_dim], bf16, isOutput=False)
    if not bias_zero:
        b1r = nc.declare_dram_parameter("b1r", [1, hid], bf16, isOutput=False)
        b2r = nc.declare_dram_parameter("b2r", [1, out_dim], bf16, isOutput=False)
        rdinvr = nc.declare_dram_parameter(
            "rdinvr", [1, nblk * P], bf16, isOutput=False
        )
    out_ext = nc.declare_dram_parameter("out", [shard, out_dim], f32, isOutput=True)

    hloc = nc.dram_tensor("hloc", [shard, hid], bf16)
    hfull = nc.dram_tensor("hfull", [n, hid], bf16, addr_space="Shared")
    h2loc = nc.dram_tensor("h2loc", [shard, P], bf16)
    h2full = nc.dram_tensor("h2full", [n, P], bf16, addr_space="Shared")

    kin = in_dim // P  # contraction tiles for layer-1 matmul

    with tile.TileContext(nc) as tc:
        with (
            tc.tile_pool(name="const", bufs=1) as cpool,
            tc.tile_pool(name="xload", bufs=2) as xpool,
            tc.tile_pool(name="idx", bufs=12) as ipool,
            tc.tile_pool(name="gath", bufs=7) as gpool,
            tc.tile_pool(name="sind", bufs=3) as spool,
            tc.tile_pool(name="blk", bufs=16) as bpool,
            tc.tile_pool(name="psagg", bufs=1, space="PSUM") as psagg,
            tc.tile_pool(name="psh2", bufs=1, space="PSUM") as psh2,
        ):
            import contextlib

            regstack = contextlib.ExitStack()
            nreg = regstack.enter_context(nc.gpsimd.register("nreg"))
            # ---- constants into SBUF
            ident_sb = cpool.tile([P, P], bf16, tag="ident")
            nc.sync.dma_start(out=ident_sb[:], in_=ident_in[:])
            w1_t = [
                cpool.tile([P, hid], bf16, tag=f"w1_{k}", name=f"w1t{k}")
                for k in range(kin)
            ]
            for k in range(kin):
                nc.sync.dma_start(out=w1_t[k][:], in_=w1[k * P : (k + 1) * P, :])
            w2_sb = cpool.tile([hid, out_dim], bf16, tag="w2")
            nc.sync.dma_start(out=w2_sb[:], in_=w2[:])
            dinvb_sb = cpool.tile([P, nblk], f32, tag="dinvb")
            nc.sync.dma_start(out=dinvb_sb[:], in_=dinvb[:])
            dinvsq_sb = cpool.tile([P, nblk], f32, tag="dinvsq")
            nc.sync.dma_start(out=dinvsq_sb[:], in_=dinvsq[:])
            dstloc_sb = cpool.tile([P, sch.ninst], bf16, tag="dstloc")
            nc.sync.dma_start(out=dstloc_sb[:], in_=dstloc[:])
            gcnt_sb = cpool.tile([1, sch.ngath], mybir.dt.int32, tag="gcnt")
            nc.sync.dma_start(out=gcnt_sb[:], in_=gcnt_in[:])
            iotar_sb = cpool.tile([P, IOTAR_K * P], bf16, tag="iotar")
            nc.sync.dma_start(out=iotar_sb[:], in_=iotar_in[:])
            if not bias_zero:
                b1_sb = cpool.tile([1, hid], bf16, tag="b1r")
                nc.sync.dma_start(out=b1_sb[:], in_=b1r[:])
                b2_sb = cpool.tile([1, out_dim], bf16, tag="b2r")
                nc.sync.dma_start(out=b2_sb[:], in_=b2r[:])
                rdinv_sb = cpool.tile([1, nblk * P], bf16, tag="rdinvr")
                nc.sync.dma_start(out=rdinv_sb[:], in_=rdinvr[:])
            # SBUF-resident tables for self-loop adds
            hres = cpool.tile([P, nblk * hid], bf16, tag="hres")
            h2self = cpool.tile([P, nblk * out_dim], bf16, tag="h2self")

            # PSUM: matmul start=True zeroes the whole 2KB bank ("zero
            # region"), so each accumulation slot owns a full bank.
            psagg_t = [
                psagg.tile([P, 4 * P], f32, tag=f"agg{i}", name=f"aggbank{i}")
                for i in range(SB_N)
            ]

            def slot_ap(si, w):
                return psagg_t[si][:, :w]

            # ---- h~ = dinv * (x @ W1), shard-local, bf16
            XGRP = 8
            for g0 in range(0, nblk, XGRP):
                g1 = min(g0 + XGRP, nblk)
                c0, c1 = g0 * P, min(g1 * P, shard)
                xt = [
                    xpool.tile([P, XGRP * P], bf16, tag=f"xt{k}", name=f"xt{k}")
                    for k in range(kin)
                ]
                for k in range(kin):
                    nc.sync.dma_start(
                        out=xt[k][:, : c1 - c0], in_=xT[k * P : (k + 1) * P, c0:c1]
                    )
                for b in range(g0, g1):
                    m = min(P, shard - b * P)
                    hp = psagg_t[b % 4]
                    for k in range(kin):
                        nc.tensor.matmul(
                            out=hp[:m, :hid],
                            lhsT=xt[k][:, b * P - c0 : b * P - c0 + m],
                            rhs=w1_t[k][:],
                            start=(k == 0),
                            stop=(k == kin - 1),
                        )
                    nc.scalar.activation(
                        out=hres[:m, b * hid : b * hid + hid],
                        in_=hp[:m, :hid],
                        func=mybir.ActivationFunctionType.Copy,
                        scale=dinvb_sb[:m, b : b + 1],
                    )
                    nc.sync.dma_start(
                        out=hloc[b * P : b * P + m, :],
                        in_=hres[:m, b * hid : b * hid + hid],
                    )

            nc.gpsimd.collective_compute(
                "AllGather",
                mybir.AluOpType.bypass,
                ins=[hloc[:]],
                outs=[hfull[:]],
                replica_groups=[list(range(NCORES))],
            )

            gq = [0]

            def run_layer(layer):
                table = hfull if layer == 1 else h2full
                w = P if layer == 1 else out_dim
                for s in range(n_sb):
                    sb = sch.sbs[s]
                    blocks = sb["blocks"]
                    slot_of = {b: i for i, b in enumerate(blocks)}
                    # self-loop add opens each block's accumulation
                    for b in blocks:
                        m = min(P, shard - b * P)
                        si = slot_of[b]
                        if layer == 1:
                            nc.tensor.matmul(
                                out=slot_ap(si, P),
                                lhsT=hres[:m, b * hid : b * hid + hid],
                                rhs=ident_sb[:m, :],
                                start=True,
                                stop=False,
                            )
                        else:
                            nc.tensor.matmul(
                                out=slot_ap(si, out_dim),
                                lhsT=ident_sb[:m, :],
                                rhs=h2self[:m, b * out_dim : (b + 1) * out_dim],
                                start=True,
                                stop=False,
                            )
                        if not bias_zero:
                            # rank-1 bias: b1 (x) 1/dinv  /  b2/dinv
                            if layer == 1:
                                nc.tensor.matmul(
                                    out=slot_ap(si, P),
                                    lhsT=b1_sb[:, :],
                                    rhs=rdinv_sb[:, b * P : (b + 1) * P],
                                    start=False,
                                    stop=False,
                                )
                            else:
                                nc.tensor.matmul(
                                    out=slot_ap(si, out_dim),
                                    lhsT=rdinv_sb[:, b * P : (b + 1) * P],
                                    rhs=b2_sb[:, :],
                                    start=False,
                                    stop=False,
                                )
                    for g in sb["gathers"]:
                        c16, nidx, k = g["c16"], g["nidx"], g["bank"]
                        gi = g["gi"]
                        nch = nidx // P
                        ninst_g = len(g["insts"])
                        it = ipool.tile(
                            [P, GMAX_CHUNKS * 8], mybir.dt.int16, tag="it"
                        )
                        nc.sync.dma_start(
                            out=it[:, : nidx // 16],
                            in_=idxs[:, c16 : c16 + nidx // 16],
                        )
                        gt = gpool.tile([P, GMAX_CHUNKS, P], bf16, tag="gt")
                        if gq[0] < 7:
                            # first use of each pool buffer: clear stale SBUF
                            # so trimmed (unwritten) slots stay finite
                            nc.vector.memset(gt[:], 0.0)
                        r0 = k * sch.bank_rows
                        r1 = min(r0 + sch.bank_rows, n)
                        nc.gpsimd.reg_load(nreg, gcnt_sb[:1, gi : gi + 1])
                        nc.gpsimd.dma_gather(
                            out_ap=gt[:, :nch, :],
                            in_ap=table[r0:r1, :],
                            idxs_ap=it[:, : nidx // 16],
                            num_idxs=nidx,
                            num_idxs_reg=nreg,
                            elem_size=P,
                            single_packet=False,
                            queue_num=gq[0] % NQUEUES,
                        )
                        gq[0] += 1
                        # batched indicator build for all instances
                        sbig = spool.tile([P, IOTAR_K, P], bf16, tag="sind")
                        nc.vector.tensor_tensor(
                            out=sbig[:, :ninst_g, :],
                            in0=iotar_sb[:, : ninst_g * P].rearrange(
                                "p (k f) -> p k f", k=ninst_g
                            ),
                            in1=dstloc_sb[
                                :, g["i0"] : g["i0"] + ninst_g
                            ].to_broadcast([P, ninst_g, P]),
                            op=mybir.AluOpType.is_equal,
                        )
                        for ii, (j, b) in enumerate(g["insts"]):
                            si = slot_of[b]
                            sp = g["stops"][ii]
                            if layer == 1:
                                nc.tensor.matmul(
                                    out=slot_ap(si, P),
                                    lhsT=gt[:, j, :],
                                    rhs=sbig[:, ii, :],
                                    start=False,
                                    stop=sp,
                                )
                            else:
                                nc.tensor.matmul(
                                    out=slot_ap(si, out_dim),
                                    lhsT=sbig[:, ii, :],
                                    rhs=gt[:, j, :out_dim],
                                    start=False,
                                    stop=sp,
                                )
                    # ---- block epilogues (Scalar + TensorE only)
                    for b in blocks:
                        m = min(P, shard - b * P)
                        si = slot_of[b]
                        if layer == 1:
                            o1 = bpool.tile([P, P], bf16, tag="o1")
                            nc.scalar.activation(
                                out=o1[:],
                                in_=slot_ap(si, P),
                                func=mybir.ActivationFunctionType.Relu,
                            )
                            h2p = psh2.tile([P, out_dim], f32, tag="h2p")
                            nc.tensor.matmul(
                                out=h2p[:m, :],
                                lhsT=o1[:, :m],
                                rhs=w2_sb[:],
                                start=True,
                                stop=True,
                            )
                            h2s = bpool.tile([P, P], bf16, tag="h2s")
                            nc.vector.memset(h2s[:, out_dim:], 0.0)
                            nc.scalar.activation(
                                out=h2s[:m, :out_dim],
                                in_=h2p[:m, :],
                                func=mybir.ActivationFunctionType.Copy,
                                scale=dinvsq_sb[:m, b : b + 1],
                            )
                            nc.scalar.activation(
                                out=h2self[:m, b * out_dim : (b + 1) * out_dim],
                                in_=h2p[:m, :],
                                func=mybir.ActivationFunctionType.Copy,
                                scale=dinvsq_sb[:m, b : b + 1],
                            )
                            nc.sync.dma_start(
                                out=h2loc[b * P : b * P + m, :], in_=h2s[:m, :]
                            )
                        else:
                            ob = bpool.tile([P, out_dim], f32, tag="ob")
                            nc.scalar.activation(
                                out=ob[:m, :],
                                in_=slot_ap(si, out_dim)[:m, :],
                                func=mybir.ActivationFunctionType.Copy,
                                scale=dinvb_sb[:m, b : b + 1],
                            )
                            nc.sync.dma_start(
                                out=out_ext[b * P : b * P + m, :], in_=ob[:m, :]
                            )

            run_layer(1)
            nc.gpsimd.collective_compute(
                "AllGather",
                mybir.AluOpType.bypass,
                ins=[h2loc[:]],
                outs=[h2full[:]],
                replica_groups=[list(range(NCORES))],
            )
            run_layer(2)
            regstack.close()

    nc.compile()
    return nc


# ---------------------------------------------------------------- kernel ---
def _make_in_maps(sch, x, W1, b1v, W2, b2v, bias_zero):
    hid = W1.shape[1]
    out_dim = W2.shape[1]
    shard = sch.shard
    nblk = sch.nblk
    bf = ml_dtypes.bfloat16
    in_maps = []
    w1b = W1.astype(bf)
    w2b = W2.astype(bf)
    iotar = np.tile(np.arange(P, dtype=np.float32), (P, IOTAR_K)).astype(bf)
    ident = np.eye(P, dtype=np.float32).astype(bf)
    for c in range(NCORES):
        xs = np.ascontiguousarray(x[c * shard : (c + 1) * shard].astype(bf).T)
        dv = sch.dinv[c * shard : (c + 1) * shard].astype(np.float64)
        full = np.zeros(nblk * P, np.float64)
        full[:shard] = dv
        cols = np.ascontiguousarray(full.reshape(nblk, P).T)
        m = {
            "xT": xs,
            "idxs": sch.idx_stream[c],
            "dstloc": sch.dstloc_s[c],
            "gcnt": sch.gcnt[c].reshape(1, -1),
            "iotar": iotar,
            "ident": ident,
            "dinvb": cols.astype(np.float32),
            "dinvsq": (cols**2).astype(np.float32),
            "W1": w1b,
            "W2": w2b,
        }
        if not bias_zero:
            rd = np.zeros(nblk * P, np.float64)
            rd[:shard] = 1.0 / dv
            m["b1r"] = b1v.reshape(1, hid).astype(bf)
            m["b2r"] = b2v.reshape(1, out_dim).astype(bf)
            m["rdinvr"] = rd.reshape(1, nblk * P).astype(bf)
        in_maps.append(m)
    return in_maps


def _get_compiled(n, e, edge_index, in_dim, hid, out_dim, bias_zero):
    key = ("nc", n, e, bias_zero)
    if key not in _CACHE:
        sch = _preprocess(n, edge_index)
        _CACHE[("sched", n, e)] = sch
        _CACHE[key] = _build(sch, in_dim, hid, out_dim, bias_zero)
    return _CACHE[("sched", n, e)], _CACHE[key]


def kernel(x, edge_index, W1, b1, W2, b2):
    _install_compat()
    from concourse.bass_utils import run_bass_kernel_spmd

    x = np.asarray(x)
    edge_index = np.asarray(edge_index)
    W1 = np.asarray(W1, np.float32)
    b1v = np.asarray(b1, np.float32)
    W2 = np.asarray(W2, np.float32)
    b2v = np.asarray(b2, np.float32)
    n, in_dim = x.shape
    hid = W1.shape[1]
    out_dim = W2.shape[1]
    bias_zero = bool(np.all(b1v == 0) and np.all(b2v == 0))

    sch, nc = _get_compiled(
        n, edge_index.shape[1], edge_index, in_dim, hid, out_dim, bias_zero
    )
    in_maps = _make_in_maps(sch, x, W1, b1v, W2, b2v, bias_zero)
    import os

    trace = bool(os.environ.get("GCN_TRACE"))
    res = run_bass_kernel_spmd(
        nc, in_maps, core_ids=list(range(NCORES)), trace=trace
    )
    global LAST_EXEC_NS
    LAST_EXEC_NS = res.exec_time_ns
    return np.concatenate([res.results[c]["out"] for c in range(NCORES)], axis=0)


LAST_EXEC_NS = None


# revision 57
# speedup vs baseline: 3.4852x; 1.0075x over previous
"""2-layer GCN (GCNConv x2) on 8 Trainium2 NeuronCores.

Strategy (dst-sharded, edge-partitioned by destination):
- Each core owns N/8 destination nodes and the edges pointing at them.
- h~ = dinv * (x @ W1) computed per-shard, AllGathered to a full bf16 table.
- Per-edge messages fetched with dma_gather (4 SWDGE queues round-robin);
  scatter-add done as one-hot-indicator matmuls accumulating in PSUM.
- Q7 descriptor generation is the critical resource, so the schedule
  minimizes gathered slots: no self-loop edges (self term is an
  identity-matmul add of SBUF-resident rows), per-(superblock, bank)
  chunking with per-block max-core slot packing, trailing -1 indices so
  each core only generates descriptors for its real edge count.
- Epilogues run on Scalar/TensorE only (relu(dinv*x) = dinv*relu(x));
  DVE just builds indicator matrices (batched is_equal per gather).
"""
import sys
import types

import numpy as np
import ml_dtypes

P = 128
NCORES = 8
GMAX_CHUNKS = 40  # max chunks (128 idxs each) per dma_gather
SB_N = 7  # dst blocks per super-block = PSUM accumulation banks
NQUEUES = 4
IOTAR_K = 48  # max instances per batched is_eq

_CACHE = {}


# ---------------------------------------------------------------- compat ---
def _install_compat():
    """Patches for this axon/walrus stack (drain waits, per-inst wait caps,
    NTFF shim). Idempotent."""
    if _CACHE.get("compat"):
        return
    import concourse.tile as tile
    import concourse.mybir as mybir

    _ev = [0]

    def _split_inst_waits(ordered):
        for _bb, insts in ordered.items():
            out = []
            for inst in insts:
                si = getattr(inst, "sync_info", None)
                if si is not None and si.on_wait is not None and len(si.on_wait) > 1:
                    waits = list(si.on_wait)
                    excess, keep = waits[:-1], waits[-1:]
                    si.on_wait.clear()
                    for sw in keep:
                        si.on_wait.append(sw)
                    for i in range(0, len(excess), 2):
                        _ev[0] += 1
                        ev = mybir.InstEventSemaphore(
                            name=f"evsplit-{_ev[0]}", ins=[], outs=[]
                        )
                        ev.engine = inst.engine
                        ev.sync_info = mybir.SyncInfo(
                            on_wait=excess[i : i + 2], on_update=[]
                        )
                        out.append(ev)
                out.append(inst)
            insts[:] = out

    orig_lower = tile.TileContext._lower_ordered_insts

    def patched_lower(self, ordered):
        _split_inst_waits(ordered)
        return orig_lower(self, ordered)

    def patched_drain(self, tick_clock, wait_clock):
        sems_alloc = list(self.sems.allocated().values())
        carrier = self.nc.sync.wait_ge(sems_alloc[0], 0)
        wait_clock.add_sem_waits(
            carrier.ins, tile.ScopedClock({None: tick_clock.global_clock})
        )
        waits = list(carrier.ins.sync_info.on_wait)
        carrier.ins.sync_info.on_wait.clear()
        for sw in waits[:2]:
            carrier.ins.sync_info.on_wait.append(sw)
        for i in range(2, len(waits), 2):
            c = self.nc.sync.wait_ge(sems_alloc[0], 0)
            c.ins.sync_info.on_wait.clear()
            for sw in waits[i : i + 2]:
                c.ins.sync_info.on_wait.append(sw)
        self.nc.sync.drain(fusable=False)
        self.nc.all_engine_barrier()
        popped = self.nc._tile_sem_poison_stack.pop()
        assert popped is self._sem_poison
        self.nc.clear_and_free_semaphores(sems_alloc)
        self.nc.all_engine_barrier()

    tile.TileContext._lower_ordered_insts = patched_lower
    tile.TileContext._drain_and_barrier = patched_drain

    # NTFF profile hook shim (missing antenv.axon_hooks in this image)
    _hook = {}
    mod = types.ModuleType("antenv.axon_hooks")
    mod.set_axon_ntff_profile_hook = lambda h: _hook.update(hook=h)
    mod.get_axon_ntff_profile_hook = lambda: _hook.get("hook")
    sys.modules["antenv.axon_hooks"] = mod
    try:
        import antenv

        antenv.axon_hooks = mod
        from trn_agent_boot.trn_boot import _ntff_profile_via_ctypes

        mod.set_axon_ntff_profile_hook(
            _ntff_profile_via_ctypes("/opt/axon/libaxon_pjrt.so")
        )
    except Exception:
        pass
    _CACHE["compat"] = True


# ---------------------------------------------------------- preprocessing ---
class Schedule:
    pass


def _round_up(v, m):
    return (v + m - 1) // m * m


def _preprocess(n, edge_index):
    """Build the uniform cross-core schedule + per-core data streams.

    Layout: for each superblock s (SB_N dst blocks), for each src bank k,
    a segment holding each block's edges padded to the max count over
    cores, chunked into 128-slot units.  Chunks may span block boundaries;
    each (chunk, block) pair is one matmul instance with its own
    indicator column stream.
    """
    shard = n // NCORES
    nblk = (shard + P - 1) // P
    nbanks = 4
    qs = shard // nbanks
    bank_rows = (n + nbanks - 1) // nbanks
    assert bank_rows <= 32767
    n_sb = (nblk + SB_N - 1) // SB_N

    src = edge_index[0].astype(np.int64)
    dst = edge_index[1].astype(np.int64)
    e = src.shape[0]
    deg = np.bincount(dst, minlength=n).astype(np.float64) + 1.0
    dinv = (1.0 / np.sqrt(deg)).astype(np.float32)

    core = dst // shard
    dl = dst - core * shard
    blk = dl // P
    dstloc = (dl % P).astype(np.int32)
    bank = src // bank_rows
    bidx = (src % bank_rows).astype(np.int32)

    # per (core, blk, bank) counts -> per-block max over cores
    cnt = np.bincount(
        (core * nblk + blk) * nbanks + bank, minlength=NCORES * nblk * nbanks
    ).reshape(NCORES, nblk, nbanks)
    mx = cnt.max(axis=0)  # [nblk, nbanks]

    # ---- compile-time program ----------------------------------------
    # gathers: (c16, nidx, bank, sb, i0, ninst) ; instances: (g, j, b, stop)
    sbs = []
    tot_slots = 0
    ninst_tot = 0
    gathers_all = []
    # chunk slot ranges per (sb, bank): block boundaries inside the segment
    seg_info = {}  # (s, k) -> (slot_base, [(b, off, mxcnt)...], nch_padded)
    for s in range(n_sb):
        blocks = list(range(s * SB_N, min((s + 1) * SB_N, nblk)))
        sb = {"blocks": blocks, "gathers": [], "instances": []}
        # last accumulate instance per block for stop flags
        for k in range(nbanks):
            seg_blocks = []
            off = 0
            for b in blocks:
                seg_blocks.append((b, off, int(mx[b, k])))
                off += int(mx[b, k])
            nseg = off
            nch_real = (nseg + P - 1) // P
            # split into gathers of <= GMAX chunks, pad each to mult of 4
            c0 = 0
            gath_list = []
            while c0 < nch_real:
                gn_real = min(GMAX_CHUNKS, nch_real - c0)
                gath_list.append((c0, gn_real, gn_real))
                c0 += gn_real
            seg_info[(s, k)] = (tot_slots, seg_blocks, gath_list)
            for (c0g, gn_real, gn) in gath_list:
                # instances: blocks overlapping chunks [c0g, c0g+gn_real)
                insts = []
                for j in range(gn_real):
                    lo = (c0g + j) * P
                    hi = lo + P
                    for (b, boff, bmx) in seg_blocks:
                        if boff < hi and boff + bmx > lo:
                            insts.append((j, b))
                g = {
                    "c16": tot_slots // 16,
                    "nidx": gn * P,
                    "bank": k,
                    "i0": ninst_tot,
                    "insts": insts,
                    "gi": len(gathers_all),
                }
                ninst_tot += len(insts)
                tot_slots += gn * P
                sb["gathers"].append(g)
                gathers_all.append(g)
        # stop flags: last instance of each block within this sb
        last = {}
        for g in sb["gathers"]:
            for idx, (j, b) in enumerate(g["insts"]):
                last[b] = (id(g), idx)
        for g in sb["gathers"]:
            g["stops"] = [
                last.get(b) == (id(g), idx) for idx, (j, b) in enumerate(g["insts"])
            ]
        sbs.append(sb)

    # ---- per-core streams ---------------------------------------------
    idx_stream = np.zeros((NCORES, 16, tot_slots // 16), np.int16)
    dstloc_s = np.full((NCORES, P, ninst_tot), -1.0, ml_dtypes.bfloat16)
    ngath = len(gathers_all)
    gcnt = np.zeros((NCORES, ngath), np.int32)

    order = np.lexsort((bidx, blk, bank, core))
    so_core = core[order]
    so_blk = blk[order]
    so_bank = bank[order]
    so_bidx = bidx[order]
    so_dstloc = dstloc[order]
    ptr = np.searchsorted(so_core, np.arange(NCORES + 1))

    for c in range(NCORES):
        lo, hi = ptr[c], ptr[c + 1]
        cblk = so_blk[lo:hi]
        cbank = so_bank[lo:hi]
        cbidx = so_bidx[lo:hi]
        cdl = so_dstloc[lo:hi]
        # run starts per (bank, blk): sorted by (bank, blk)
        key = cbank * nblk + cblk
        idx_flat = np.full(tot_slots, -1, np.int32)
        dl_flat = np.full(tot_slots, -1, np.int32)
        blk_flat = np.full(tot_slots, -2, np.int32)
        real = np.zeros(tot_slots, bool)
        for s in range(n_sb):
            for k in range(nbanks):
                slot_base, seg_blocks, gath_list = seg_info[(s, k)]
                for (b, boff, bmx) in seg_blocks:
                    want = k * nblk + b
                    p0 = np.searchsorted(key, want, "left")
                    p1 = np.searchsorted(key, want, "right")
                    cn = p1 - p0
                    assert cn <= bmx
                    sl = slot_base + boff
                    idx_flat[sl : sl + cn] = cbidx[p0:p1]
                    dl_flat[sl : sl + cn] = cdl[p0:p1]
                    blk_flat[sl : sl + cn] = b
                    real[sl : sl + cn] = True
        # per-gather: pads before the last real slot gather row 0 (harmless),
        # pads after are trailing -1 (skipped by Q7 desc-gen; count goes in
        # the per-core num_idxs register).
        for gi, g in enumerate(gathers_all):
            c16, nidx = g["c16"], g["nidx"]
            w0 = c16 * 16
            rw = real[w0 : w0 + nidx]
            nz = np.nonzero(rw)[0]
            last = int(nz[-1]) + 1 if len(nz) else 0
            gcnt[c, gi] = last
            w = idx_flat[w0 : w0 + nidx]
            w[:last][~rw[:last]] = 0
            w[last:] = -1
            idx_stream[c][:, c16 : c16 + nidx // 16] = (
                w.reshape(nidx // 16, 16).T.astype(np.int16)
            )
            # instance dstloc columns
            for ii, (j, b) in enumerate(g["insts"]):
                cslots = slice(c16 * 16 + j * P, c16 * 16 + (j + 1) * P)
                m = blk_flat[cslots] == b
                col = np.where(m, dl_flat[cslots], -1).astype(np.float32)
                dstloc_s[c][:, g["i0"] + ii] = col.astype(ml_dtypes.bfloat16)

    sch = Schedule()
    sch.n, sch.e, sch.shard, sch.nblk, sch.nbanks = n, e, shard, nblk, nbanks
    sch.bank_rows, sch.n_sb, sch.qs = bank_rows, n_sb, qs
    sch.tot_slots = tot_slots
    sch.ninst = ninst_tot
    sch.ngath = ngath
    sch.gcnt = gcnt
    sch.sbs = sbs
    sch.dinv = dinv
    sch.idx_stream = np.tile(idx_stream, (1, 8, 1))  # replicate to 128 parts
    sch.dstloc_s = dstloc_s
    return sch


# ----------------------------------------------------------------- build ---
def _build(sch, in_dim, hid, out_dim, bias_zero):
    import concourse.mybir as mybir
    import concourse.tile as tile
    from concourse import bacc

    bf16 = mybir.dt.bfloat16
    f32 = mybir.dt.float32
    shard, nblk, nbanks = sch.shard, sch.nblk, sch.nbanks
    n_sb = sch.n_sb
    n = sch.n

    nc = bacc.Bacc(num_swdge_queues=NQUEUES, dynamic_dma_scratch_size=32768)

    xT = nc.declare_dram_parameter("xT", [in_dim, shard], bf16, isOutput=False)
    idxs = nc.declare_dram_parameter(
        "idxs", [P, sch.tot_slots // 16], mybir.dt.int16, isOutput=False
    )
    dstloc = nc.declare_dram_parameter("dstloc", [P, sch.ninst], bf16, isOutput=False)
    gcnt_in = nc.declare_dram_parameter(
        "gcnt", [1, sch.ngath], mybir.dt.int32, isOutput=False
    )
    iotar_in = nc.declare_dram_parameter(
        "iotar", [P, IOTAR_K * P], bf16, isOutput=False
    )
    ident_in = nc.declare_dram_parameter("ident", [P, P], bf16, isOutput=False)
    dinvb = nc.declare_dram_parameter("dinvb", [P, nblk], f32, isOutput=False)
    dinvsq = nc.declare_dram_parameter("dinvsq", [P, nblk], f32, isOutput=False)
    w1 = nc.declare_dram_parameter("W1", [in_dim, hid], bf16, isOutput=False)
    w2 = nc.declare_dram_parameter("W2", [hid, out_dim], bf16, isOutput=False)
    if not bias_zero:
        b1r = nc.declare_dram_parameter("b1r", [1, hid], bf16, isOutput=False)
        b2r = nc.declare_dram_parameter("b2r", [1, out_dim], bf16, isOutput=False)
        rdinvr = nc.declare_dram_parameter(
            "rdinvr", [1, nblk * P], bf16, isOutput=False
        )
    out_ext = nc.declare_dram_parameter("out", [shard, out_dim], f32, isOutput=True)

    hloc = nc.dram_tensor("hloc", [shard, hid], bf16)
    hfull = nc.dram_tensor("hfull", [n, hid], bf16, addr_space="Shared")
    h2loc = nc.dram_tensor("h2loc", [shard, P], bf16)
    h2full = nc.dram_tensor("h2full", [n, P], bf16, addr_space="Shared")

    kin = in_dim // P  # contraction tiles for layer-1 matmul

    with tile.TileContext(nc) as tc:
        with (
            tc.tile_pool(name="const", bufs=1) as cpool,
            tc.tile_pool(name="xload", bufs=2) as xpool,
            tc.tile_pool(name="idx", bufs=12) as ipool,
            tc.tile_pool(name="gath", bufs=7) as gpool,
            tc.tile_pool(name="sind", bufs=3) as spool,
            tc.tile_pool(name="blk", bufs=16) as bpool,
            tc.tile_pool(name="psagg", bufs=1, space="PSUM") as psagg,
            tc.tile_pool(name="psh2", bufs=1, space="PSUM") as psh2,
        ):
            import contextlib

            regstack = contextlib.ExitStack()
            nreg = regstack.enter_context(nc.gpsimd.register("nreg"))
            # ---- constants into SBUF
            ident_sb = cpool.tile([P, P], bf16, tag="ident")
            nc.sync.dma_start(out=ident_sb[:], in_=ident_in[:])
            w1_t = [
                cpool.tile([P, hid], bf16, tag=f"w1_{k}", name=f"w1t{k}")
                for k in range(kin)
            ]
            for k in range(kin):
                nc.sync.dma_start(out=w1_t[k][:], in_=w1[k * P : (k + 1) * P, :])
            w2_sb = cpool.tile([hid, out_dim], bf16, tag="w2")
            nc.sync.dma_start(out=w2_sb[:], in_=w2[:])
            dinvb_sb = cpool.tile([P, nblk], f32, tag="dinvb")
            nc.sync.dma_start(out=dinvb_sb[:], in_=dinvb[:])
            dinvsq_sb = cpool.tile([P, nblk], f32, tag="dinvsq")
            nc.sync.dma_start(out=dinvsq_sb[:], in_=dinvsq[:])
            dstloc_sb = cpool.tile([P, sch.ninst], bf16, tag="dstloc")
            nc.sync.dma_start(out=dstloc_sb[:], in_=dstloc[:])
            gcnt_sb = cpool.tile([1, sch.ngath], mybir.dt.int32, tag="gcnt")
            nc.sync.dma_start(out=gcnt_sb[:], in_=gcnt_in[:])
            iotar_sb = cpool.tile([P, IOTAR_K * P], bf16, tag="iotar")
            nc.sync.dma_start(out=iotar_sb[:], in_=iotar_in[:])
            if not bias_zero:
                b1_sb = cpool.tile([1, hid], bf16, tag="b1r")
                nc.sync.dma_start(out=b1_sb[:], in_=b1r[:])
                b2_sb = cpool.tile([1, out_dim], bf16, tag="b2r")
                nc.sync.dma_start(out=b2_sb[:], in_=b2r[:])
                rdinv_sb = cpool.tile([1, nblk * P], bf16, tag="rdinvr")
                nc.sync.dma_start(out=rdinv_sb[:], in_=rdinvr[:])
            # SBUF-resident tables for self-loop adds
            hres = cpool.tile([P, nblk * hid], bf16, tag="hres")
            h2self = cpool.tile([P, nblk * out_dim], bf16, tag="h2self")

            # PSUM: matmul start=True zeroes the whole 2KB bank ("zero
            # region"), so each accumulation slot owns a full bank.
            psagg_t = [
                psagg.tile([P, 4 * P], f32, tag=f"agg{i}", name=f"aggbank{i}")
                for i in range(SB_N)
            ]

            def slot_ap(si, w):
                return psagg_t[si][:, :w]

            # ---- h~ = dinv * (x @ W1), shard-local, bf16
            XGRP = 8
            for g0 in range(0, nblk, XGRP):
                g1 = min(g0 + XGRP, nblk)
                c0, c1 = g0 * P, min(g1 * P, shard)
                xt = [
                    xpool.tile([P, XGRP * P], bf16, tag=f"xt{k}", name=f"xt{k}")
                    for k in range(kin)
                ]
                for k in range(kin):
                    nc.sync.dma_start(
                        out=xt[k][:, : c1 - c0], in_=xT[k * P : (k + 1) * P, c0:c1]
                    )
                for b in range(g0, g1):
                    m = min(P, shard - b * P)
                    hp = psagg_t[b % 4]
                    for k in range(kin):
                        nc.tensor.matmul(
                            out=hp[:m, :hid],
                            lhsT=xt[k][:, b * P - c0 : b * P - c0 + m],
                            rhs=w1_t[k][:],
                            start=(k == 0),
                            stop=(k == kin - 1),
                        )
                    nc.scalar.activation(
                        out=hres[:m, b * hid : b * hid + hid],
                        in_=hp[:m, :hid],
                        func=mybir.ActivationFunctionType.Copy,
                        scale=dinvb_sb[:m, b : b + 1],
                    )
                    nc.sync.dma_start(
                        out=hloc[b * P : b * P + m, :],
                        in_=hres[:m, b * hid : b * hid + hid],
                    )

            nc.gpsimd.collective_compute(
                "AllGather",
                mybir.AluOpType.bypass,
                ins=[hloc[:]],
                outs=[hfull[:]],
                replica_groups=[list(range(NCORES))],
            )

            gq = [0]

            def run_layer(layer):
                table = hfull if layer == 1 else h2full
                w = P if layer == 1 else out_dim
                for s in range(n_sb):
                    sb = sch.sbs[s]
                    blocks = sb["blocks"]
                    slot_of = {b: i for i, b in enumerate(blocks)}
                    # self-loop add opens each block's accumulation
                    for b in blocks:
                        m = min(P, shard - b * P)
                        si = slot_of[b]
                        if layer == 1:
                            nc.tensor.matmul(
                                out=slot_ap(si, P),
                                lhsT=hres[:m, b * hid : b * hid + hid],
                                rhs=ident_sb[:m, :],
                                start=True,
                                stop=False,
                            )
                        else:
                            nc.tensor.matmul(
                                out=slot_ap(si, out_dim),
                                lhsT=ident_sb[:m, :],
                                rhs=h2self[:m, b * out_dim : (b + 1) * out_dim],
                                start=True,
                                stop=False,
                            )
                        if not bias_zero:
                            # rank-1 bias: b1 (x) 1/dinv  /  b2/dinv
                            if layer == 1:
                                nc.tensor.matmul(
                                    out=slot_ap(si, P),
                                    lhsT=b1_sb[:, :],
                                    rhs=rdinv_sb[:, b * P : (b + 1) * P],
                                    start=False,
                                    stop=False,
                                )
                            else:
                                nc.tensor.matmul(
                                    out=slot_ap(si, out_dim),
                                    lhsT=rdinv_sb[:, b * P : (b + 1) * P],
                                    rhs=b2_sb[:, :],
                                    start=False,
                                    stop=False,
                                )
                    for g in sb["gathers"]:
                        c16, nidx, k = g["c16"], g["nidx"], g["bank"]
                        gi = g["gi"]
                        nch = nidx // P
                        ninst_g = len(g["insts"])
                        it = ipool.tile(
                            [P, GMAX_CHUNKS * 8], mybir.dt.int16, tag="it"
                        )
                        nc.sync.dma_start(
                            out=it[:, : nidx // 16],
                            in_=idxs[:, c16 : c16 + nidx // 16],
                        )
                        gt = gpool.tile([P, GMAX_CHUNKS, P], bf16, tag="gt")
                        if gq[0] < 7:
                            # first use of each pool buffer: clear stale SBUF
                            # so trimmed (unwritten) slots stay finite
                            nc.vector.memset(gt[:], 0.0)
                        r0 = k * sch.bank_rows
                        r1 = min(r0 + sch.bank_rows, n)
                        nc.gpsimd.reg_load(nreg, gcnt_sb[:1, gi : gi + 1])
                        nc.gpsimd.dma_gather(
                            out_ap=gt[:, :nch, :],
                            in_ap=table[r0:r1, :],
                            idxs_ap=it[:, : nidx // 16],
                            num_idxs=nidx,
                            num_idxs_reg=nreg,
                            elem_size=P,
                            single_packet=False,
                            queue_num=gq[0] % NQUEUES,
                        )
                        gq[0] += 1
                        # batched indicator build for all instances
                        sbig = spool.tile([P, IOTAR_K, P], bf16, tag="sind")
                        nc.vector.tensor_tensor(
                            out=sbig[:, :ninst_g, :],
                            in0=iotar_sb[:, : ninst_g * P].rearrange(
                                "p (k f) -> p k f", k=ninst_g
                            ),
                            in1=dstloc_sb[
                                :, g["i0"] : g["i0"] + ninst_g
                            ].to_broadcast([P, ninst_g, P]),
                            op=mybir.AluOpType.is_equal,
                        )
                        for ii, (j, b) in enumerate(g["insts"]):
                            si = slot_of[b]
                            sp = g["stops"][ii]
                            if layer == 1:
                                nc.tensor.matmul(
                                    out=slot_ap(si, P),
                                    lhsT=gt[:, j, :],
                                    rhs=sbig[:, ii, :],
                                    start=False,
                                    stop=sp,
                                )
                            else:
                                nc.tensor.matmul(
                                    out=slot_ap(si, out_dim),
                                    lhsT=sbig[:, ii, :],
                                    rhs=gt[:, j, :out_dim],
                                    start=False,
                                    stop=sp,
                                )
                    # ---- block epilogues (Scalar + TensorE only)
                    # L1: all relus first — each relu is the PSUM read that
                    # frees its slot; keeping them ahead of the (matmul-gated)
                    # h2s/h2self copies in the scalar queue releases all
                    # slots for the next superblock immediately.
                    o1_t = {}
                    if layer == 1:
                        for b in blocks:
                            o1 = bpool.tile([P, P], bf16, tag="o1")
                            nc.scalar.activation(
                                out=o1[:],
                                in_=slot_ap(slot_of[b], P),
                                func=mybir.ActivationFunctionType.Relu,
                            )
                            o1_t[b] = o1
                    for b in blocks:
                        m = min(P, shard - b * P)
                        si = slot_of[b]
                        if layer == 1:
                            o1 = o1_t[b]
                            h2p = psh2.tile([P, out_dim], f32, tag="h2p")
                            nc.tensor.matmul(
                                out=h2p[:m, :],
                                lhsT=o1[:, :m],
                                rhs=w2_sb[:],
                                start=True,
                                stop=True,
                            )
                            h2s = bpool.tile([P, P], bf16, tag="h2s")
                            nc.vector.memset(h2s[:, out_dim:], 0.0)
                            nc.scalar.activation(
                                out=h2s[:m, :out_dim],
                                in_=h2p[:m, :],
                                func=mybir.ActivationFunctionType.Copy,
                                scale=dinvsq_sb[:m, b : b + 1],
                            )
                            nc.scalar.activation(
                                out=h2self[:m, b * out_dim : (b + 1) * out_dim],
                                in_=h2p[:m, :],
                                func=mybir.ActivationFunctionType.Copy,
                                scale=dinvsq_sb[:m, b : b + 1],
                            )
                            nc.sync.dma_start(
                                out=h2loc[b * P : b * P + m, :], in_=h2s[:m, :]
                            )
                        else:
                            ob = bpool.tile([P, out_dim], f32, tag="ob")
                            nc.scalar.activation(
                                out=ob[:m, :],
                                in_=slot_ap(si, out_dim)[:m, :],
                                func=mybir.ActivationFunctionType.Copy,
                                scale=dinvb_sb[:m, b : b + 1],
                            )
                            nc.sync.dma_start(
                                out=out_ext[b * P : b * P + m, :], in_=ob[:m, :]
                            )

            run_layer(1)
            nc.gpsimd.collective_compute(
                "AllGather",
                mybir.AluOpType.bypass,
                ins=[h2loc[:]],
                outs=[h2full[:]],
                replica_groups=[list(range(NCORES))],
            )
            run_layer(2)
            regstack.close()

    nc.compile()
    return nc


# ---------------------------------------------------------------- kernel ---
def _make_in_maps(sch, x, W1, b1v, W2, b2v, bias_zero):
    hid = W1.shape[1]
    out_dim = W2.shape[1]
    shard = sch.shard
    nblk = sch.nblk
    bf = ml_dtypes.bfloat16
    in_maps = []
    w1b = W1.astype(bf)
    w2b = W2.astype(bf)
    iotar = np.tile(np.arange(P, dtype=np.float32), (P, IOTAR_K)).astype(bf)
    ident = np.eye(P, dtype=np.float32).astype(bf)
    for c in range(NCORES):
        xs = np.ascontiguousarray(x[c * shard : (c + 1) * shard].astype(bf).T)
        dv = sch.dinv[c * shard : (c + 1) * shard].astype(np.float64)
        full = np.zeros(nblk * P, np.float64)
        full[:shard] = dv
        cols = np.ascontiguousarray(full.reshape(nblk, P).T)
        m = {
            "xT": xs,
            "idxs": sch.idx_stream[c],
            "dstloc": sch.dstloc_s[c],
            "gcnt": sch.gcnt[c].reshape(1, -1),
            "iotar": iotar,
            "ident": ident,
            "dinvb": cols.astype(np.float32),
            "dinvsq": (cols**2).astype(np.float32),
            "W1": w1b,
            "W2": w2b,
        }
        if not bias_zero:
            rd = np.zeros(nblk * P, np.float64)
            rd[:shard] = 1.0 / dv
            m["b1r"] = b1v.reshape(1, hid).astype(bf)
            m["b2r"] = b2v.reshape(1, out_dim).astype(bf)
            m["rdinvr"] = rd.reshape(1, nblk * P).astype(bf)
        in_maps.append(m)
    return in_maps


def _get_compiled(n, e, edge_index, in_dim, hid, out_dim, bias_zero):
    key = ("nc", n, e, bias_zero)
    if key not in _CACHE:
        sch = _preprocess(n, edge_index)
        _CACHE[("sched", n, e)] = sch
        _CACHE[key] = _build(sch, in_dim, hid, out_dim, bias_zero)
    return _CACHE[("sched", n, e)], _CACHE[key]


def kernel(x, edge_index, W1, b1, W2, b2):
    _install_compat()
    from concourse.bass_utils import run_bass_kernel_spmd

    x = np.asarray(x)
    edge_index = np.asarray(edge_index)
    W1 = np.asarray(W1, np.float32)
    b1v = np.asarray(b1, np.float32)
    W2 = np.asarray(W2, np.float32)
    b2v = np.asarray(b2, np.float32)
    n, in_dim = x.shape
    hid = W1.shape[1]
    out_dim = W2.shape[1]
    bias_zero = bool(np.all(b1v == 0) and np.all(b2v == 0))

    sch, nc = _get_compiled(
        n, edge_index.shape[1], edge_index, in_dim, hid, out_dim, bias_zero
    )
    in_maps = _make_in_maps(sch, x, W1, b1v, W2, b2v, bias_zero)
    import os

    trace = bool(os.environ.get("GCN_TRACE"))
    res = run_bass_kernel_spmd(
        nc, in_maps, core_ids=list(range(NCORES)), trace=trace
    )
    global LAST_EXEC_NS
    LAST_EXEC_NS = res.exec_time_ns
    return np.concatenate([res.results[c]["out"] for c in range(NCORES)], axis=0)


LAST_EXEC_NS = None


# revision 59
# speedup vs baseline: 3.5595x; 1.0213x over previous
"""2-layer GCN (GCNConv x2) on 8 Trainium2 NeuronCores.

Strategy (dst-sharded, edge-partitioned by destination):
- Each core owns N/8 destination nodes and the edges pointing at them.
- h~ = dinv * (x @ W1) computed per-shard, AllGathered to a full bf16 table.
- Per-edge messages fetched with dma_gather (4 SWDGE queues round-robin);
  scatter-add done as one-hot-indicator matmuls accumulating in PSUM.
- Q7 descriptor generation is the critical resource, so the schedule
  minimizes gathered slots: no self-loop edges (self term is an
  identity-matmul add of SBUF-resident rows), per-(superblock, bank)
  chunking with per-block max-core slot packing, trailing -1 indices so
  each core only generates descriptors for its real edge count.
- Epilogues run on Scalar/TensorE only (relu(dinv*x) = dinv*relu(x));
  DVE just builds indicator matrices (batched is_equal per gather).
"""
import sys
import types

import numpy as np
import ml_dtypes

P = 128
NCORES = 8
GMAX_CHUNKS = 40  # max chunks (128 idxs each) per dma_gather
SB_N = 7  # dst blocks per super-block = PSUM accumulation banks
NQUEUES = 4
IOTAR_K = 48  # max instances per batched is_eq

_CACHE = {}


# ---------------------------------------------------------------- compat ---
def _install_compat():
    """Patches for this axon/walrus stack (drain waits, per-inst wait caps,
    NTFF shim). Idempotent."""
    if _CACHE.get("compat"):
        return
    import concourse.tile as tile
    import concourse.mybir as mybir

    _ev = [0]

    def _split_inst_waits(ordered):
        for _bb, insts in ordered.items():
            out = []
            for inst in insts:
                si = getattr(inst, "sync_info", None)
                if si is not None and si.on_wait is not None and len(si.on_wait) > 1:
                    waits = list(si.on_wait)
                    excess, keep = waits[:-1], waits[-1:]
                    si.on_wait.clear()
                    for sw in keep:
                        si.on_wait.append(sw)
                    for i in range(0, len(excess), 2):
                        _ev[0] += 1
                        ev = mybir.InstEventSemaphore(
                            name=f"evsplit-{_ev[0]}", ins=[], outs=[]
                        )
                        ev.engine = inst.engine
                        ev.sync_info = mybir.SyncInfo(
                            on_wait=excess[i : i + 2], on_update=[]
                        )
                        out.append(ev)
                out.append(inst)
            insts[:] = out

    orig_lower = tile.TileContext._lower_ordered_insts

    def patched_lower(self, ordered):
        _split_inst_waits(ordered)
        return orig_lower(self, ordered)

    def patched_drain(self, tick_clock, wait_clock):
        sems_alloc = list(self.sems.allocated().values())
        carrier = self.nc.sync.wait_ge(sems_alloc[0], 0)
        wait_clock.add_sem_waits(
            carrier.ins, tile.ScopedClock({None: tick_clock.global_clock})
        )
        waits = list(carrier.ins.sync_info.on_wait)
        carrier.ins.sync_info.on_wait.clear()
        for sw in waits[:2]:
            carrier.ins.sync_info.on_wait.append(sw)
        for i in range(2, len(waits), 2):
            c = self.nc.sync.wait_ge(sems_alloc[0], 0)
            c.ins.sync_info.on_wait.clear()
            for sw in waits[i : i + 2]:
                c.ins.sync_info.on_wait.append(sw)
        self.nc.sync.drain(fusable=False)
        self.nc.all_engine_barrier()
        popped = self.nc._tile_sem_poison_stack.pop()
        assert popped is self._sem_poison
        self.nc.clear_and_free_semaphores(sems_alloc)
        self.nc.all_engine_barrier()

    tile.TileContext._lower_ordered_insts = patched_lower
    tile.TileContext._drain_and_barrier = patched_drain

    # NTFF profile hook shim (missing antenv.axon_hooks in this image)
    _hook = {}
    mod = types.ModuleType("antenv.axon_hooks")
    mod.set_axon_ntff_profile_hook = lambda h: _hook.update(hook=h)
    mod.get_axon_ntff_profile_hook = lambda: _hook.get("hook")
    sys.modules["antenv.axon_hooks"] = mod
    try:
        import antenv

        antenv.axon_hooks = mod
        from trn_agent_boot.trn_boot import _ntff_profile_via_ctypes

        mod.set_axon_ntff_profile_hook(
            _ntff_profile_via_ctypes("/opt/axon/libaxon_pjrt.so")
        )
    except Exception:
        pass
    _CACHE["compat"] = True


# ---------------------------------------------------------- preprocessing ---
class Schedule:
    pass


def _round_up(v, m):
    return (v + m - 1) // m * m


def _preprocess(n, edge_index):
    """Build the uniform cross-core schedule + per-core data streams.

    Layout: for each superblock s (SB_N dst blocks), for each src bank k,
    a segment holding each block's edges padded to the max count over
    cores, chunked into 128-slot units.  Chunks may span block boundaries;
    each (chunk, block) pair is one matmul instance with its own
    indicator column stream.
    """
    shard = n // NCORES
    nblk = (shard + P - 1) // P
    nbanks = 4
    qs = shard // nbanks
    bank_rows = (n + nbanks - 1) // nbanks
    assert bank_rows <= 32767
    n_sb = (nblk + SB_N - 1) // SB_N

    src = edge_index[0].astype(np.int64)
    dst = edge_index[1].astype(np.int64)
    e = src.shape[0]
    deg = np.bincount(dst, minlength=n).astype(np.float64) + 1.0
    dinv = (1.0 / np.sqrt(deg)).astype(np.float32)

    core = dst // shard
    dl = dst - core * shard
    blk = dl // P
    dstloc = (dl % P).astype(np.int32)
    bank = src // bank_rows
    bidx = (src % bank_rows).astype(np.int32)

    # per (core, blk, bank) counts -> per-block max over cores
    cnt = np.bincount(
        (core * nblk + blk) * nbanks + bank, minlength=NCORES * nblk * nbanks
    ).reshape(NCORES, nblk, nbanks)
    mx = cnt.max(axis=0)  # [nblk, nbanks]

    # ---- compile-time program ----------------------------------------
    # gathers: (c16, nidx, bank, sb, i0, ninst) ; instances: (g, j, b, stop)
    sbs = []
    tot_slots = 0
    ninst_tot = 0
    gathers_all = []
    # chunk slot ranges per (sb, bank): block boundaries inside the segment
    seg_info = {}  # (s, k) -> (slot_base, [(b, off, mxcnt)...], nch_padded)
    for s in range(n_sb):
        blocks = list(range(s * SB_N, min((s + 1) * SB_N, nblk)))
        sb = {"blocks": blocks, "gathers": [], "instances": []}
        # last accumulate instance per block for stop flags
        for k in range(nbanks):
            seg_blocks = []
            off = 0
            for b in blocks:
                seg_blocks.append((b, off, int(mx[b, k])))
                off += int(mx[b, k])
            nseg = off
            nch_real = (nseg + P - 1) // P
            # split into gathers of <= GMAX chunks, pad each to mult of 4
            c0 = 0
            gath_list = []
            while c0 < nch_real:
                gn_real = min(GMAX_CHUNKS, nch_real - c0)
                gath_list.append((c0, gn_real, gn_real))
                c0 += gn_real
            seg_info[(s, k)] = (tot_slots, seg_blocks, gath_list)
            for (c0g, gn_real, gn) in gath_list:
                # instances: blocks overlapping chunks [c0g, c0g+gn_real)
                insts = []
                for j in range(gn_real):
                    lo = (c0g + j) * P
                    hi = lo + P
                    for (b, boff, bmx) in seg_blocks:
                        if boff < hi and boff + bmx > lo:
                            insts.append((j, b))
                g = {
                    "c16": tot_slots // 16,
                    "nidx": gn * P,
                    "bank": k,
                    "i0": ninst_tot,
                    "insts": insts,
                    "gi": len(gathers_all),
                }
                ninst_tot += len(insts)
                tot_slots += gn * P
                sb["gathers"].append(g)
                gathers_all.append(g)
        # stop flags: last instance of each block within this sb
        last = {}
        for g in sb["gathers"]:
            for idx, (j, b) in enumerate(g["insts"]):
                last[b] = (id(g), idx)
        for g in sb["gathers"]:
            g["stops"] = [
                last.get(b) == (id(g), idx) for idx, (j, b) in enumerate(g["insts"])
            ]
        sbs.append(sb)

    # ---- per-core streams ---------------------------------------------
    idx_stream = np.zeros((NCORES, 16, tot_slots // 16), np.int16)
    dstloc_s = np.full((NCORES, P, ninst_tot), -1.0, ml_dtypes.bfloat16)
    ngath = len(gathers_all)
    gcnt = np.zeros((NCORES, ngath), np.int32)

    order = np.lexsort((bidx, blk, bank, core))
    so_core = core[order]
    so_blk = blk[order]
    so_bank = bank[order]
    so_bidx = bidx[order]
    so_dstloc = dstloc[order]
    ptr = np.searchsorted(so_core, np.arange(NCORES + 1))

    for c in range(NCORES):
        lo, hi = ptr[c], ptr[c + 1]
        cblk = so_blk[lo:hi]
        cbank = so_bank[lo:hi]
        cbidx = so_bidx[lo:hi]
        cdl = so_dstloc[lo:hi]
        # run starts per (bank, blk): sorted by (bank, blk)
        key = cbank * nblk + cblk
        idx_flat = np.full(tot_slots, -1, np.int32)
        dl_flat = np.full(tot_slots, -1, np.int32)
        blk_flat = np.full(tot_slots, -2, np.int32)
        real = np.zeros(tot_slots, bool)
        for s in range(n_sb):
            for k in range(nbanks):
                slot_base, seg_blocks, gath_list = seg_info[(s, k)]
                for (b, boff, bmx) in seg_blocks:
                    want = k * nblk + b
                    p0 = np.searchsorted(key, want, "left")
                    p1 = np.searchsorted(key, want, "right")
                    cn = p1 - p0
                    assert cn <= bmx
                    sl = slot_base + boff
                    idx_flat[sl : sl + cn] = cbidx[p0:p1]
                    dl_flat[sl : sl + cn] = cdl[p0:p1]
                    blk_flat[sl : sl + cn] = b
                    real[sl : sl + cn] = True
        # per-gather: pads before the last real slot gather row 0 (harmless),
        # pads after are trailing -1 (skipped by Q7 desc-gen; count goes in
        # the per-core num_idxs register).
        for gi, g in enumerate(gathers_all):
            c16, nidx = g["c16"], g["nidx"]
            w0 = c16 * 16
            rw = real[w0 : w0 + nidx]
            nz = np.nonzero(rw)[0]
            last = int(nz[-1]) + 1 if len(nz) else 0
            gcnt[c, gi] = last
            w = idx_flat[w0 : w0 + nidx]
            w[:last][~rw[:last]] = 0
            w[last:] = -1
            idx_stream[c][:, c16 : c16 + nidx // 16] = (
                w.reshape(nidx // 16, 16).T.astype(np.int16)
            )
            # instance dstloc columns
            for ii, (j, b) in enumerate(g["insts"]):
                cslots = slice(c16 * 16 + j * P, c16 * 16 + (j + 1) * P)
                m = blk_flat[cslots] == b
                col = np.where(m, dl_flat[cslots], -1).astype(np.float32)
                dstloc_s[c][:, g["i0"] + ii] = col.astype(ml_dtypes.bfloat16)

    sch = Schedule()
    sch.n, sch.e, sch.shard, sch.nblk, sch.nbanks = n, e, shard, nblk, nbanks
    sch.bank_rows, sch.n_sb, sch.qs = bank_rows, n_sb, qs
    sch.tot_slots = tot_slots
    sch.ninst = ninst_tot
    sch.ngath = ngath
    sch.gcnt = gcnt
    sch.sbs = sbs
    sch.dinv = dinv
    sch.idx_stream = np.tile(idx_stream, (1, 8, 1))  # replicate to 128 parts
    sch.dstloc_s = dstloc_s
    return sch


# ----------------------------------------------------------------- build ---
def _build(sch, in_dim, hid, out_dim, bias_zero):
    import concourse.mybir as mybir
    import concourse.tile as tile
    from concourse import bacc

    bf16 = mybir.dt.bfloat16
    f32 = mybir.dt.float32
    shard, nblk, nbanks = sch.shard, sch.nblk, sch.nbanks
    n_sb = sch.n_sb
    n = sch.n

    nc = bacc.Bacc(num_swdge_queues=NQUEUES, dynamic_dma_scratch_size=32768)

    xT = nc.declare_dram_parameter("xT", [in_dim, shard], bf16, isOutput=False)
    idxs = nc.declare_dram_parameter(
        "idxs", [P, sch.tot_slots // 16], mybir.dt.int16, isOutput=False
    )
    dstloc = nc.declare_dram_parameter("dstloc", [P, sch.ninst], bf16, isOutput=False)
    gcnt_in = nc.declare_dram_parameter(
        "gcnt", [1, sch.ngath], mybir.dt.int32, isOutput=False
    )
    iotar_in = nc.declare_dram_parameter(
        "iotar", [P, IOTAR_K * P], bf16, isOutput=False
    )
    ident_in = nc.declare_dram_parameter("ident", [P, P], bf16, isOutput=False)
    dinvb = nc.declare_dram_parameter("dinvb", [P, nblk], f32, isOutput=False)
    dinvsq = nc.declare_dram_parameter("dinvsq", [P, nblk], f32, isOutput=False)
    w1 = nc.declare_dram_parameter("W1", [in_dim, hid], bf16, isOutput=False)
    w2 = nc.declare_dram_parameter("W2", [hid, out_dim], bf16, isOutput=False)
    if not bias_zero:
        b1r = nc.declare_dram_parameter("b1r", [1, hid], bf16, isOutput=False)
        b2r = nc.declare_dram_parameter("b2r", [1, out_dim], bf16, isOutput=False)
        rdinvr = nc.declare_dram_parameter(
            "rdinvr", [1, nblk * P], bf16, isOutput=False
        )
    out_ext = nc.declare_dram_parameter("out", [shard, out_dim], f32, isOutput=True)

    hloc = nc.dram_tensor("hloc", [shard, hid], bf16)
    hfull = nc.dram_tensor("hfull", [n, hid], bf16, addr_space="Shared")
    h2loc = nc.dram_tensor("h2loc", [shard, P], bf16)
    h2full = nc.dram_tensor("h2full", [n, P], bf16, addr_space="Shared")

    kin = in_dim // P  # contraction tiles for layer-1 matmul

    with tile.TileContext(nc) as tc:
        with (
            tc.tile_pool(name="const", bufs=1) as cpool,
            tc.tile_pool(name="xload", bufs=2) as xpool,
            tc.tile_pool(name="idx", bufs=12) as ipool,
            tc.tile_pool(name="gath", bufs=7) as gpool,
            tc.tile_pool(name="sind", bufs=3) as spool,
            tc.tile_pool(name="blk", bufs=16) as bpool,
            tc.tile_pool(name="psagg", bufs=1, space="PSUM") as psagg,
            tc.tile_pool(name="psh2", bufs=1, space="PSUM") as psh2,
        ):
            import contextlib

            regstack = contextlib.ExitStack()
            nreg = regstack.enter_context(nc.gpsimd.register("nreg"))
            # ---- constants into SBUF
            ident_sb = cpool.tile([P, P], bf16, tag="ident")
            nc.sync.dma_start(out=ident_sb[:], in_=ident_in[:])
            w1_t = [
                cpool.tile([P, hid], bf16, tag=f"w1_{k}", name=f"w1t{k}")
                for k in range(kin)
            ]
            for k in range(kin):
                nc.sync.dma_start(out=w1_t[k][:], in_=w1[k * P : (k + 1) * P, :])
            w2_sb = cpool.tile([hid, out_dim], bf16, tag="w2")
            nc.sync.dma_start(out=w2_sb[:], in_=w2[:])
            dinvb_sb = cpool.tile([P, nblk], f32, tag="dinvb")
            nc.sync.dma_start(out=dinvb_sb[:], in_=dinvb[:])
            dinvsq_sb = cpool.tile([P, nblk], f32, tag="dinvsq")
            nc.sync.dma_start(out=dinvsq_sb[:], in_=dinvsq[:])
            dstloc_sb = cpool.tile([P, sch.ninst], bf16, tag="dstloc")
            nc.sync.dma_start(out=dstloc_sb[:], in_=dstloc[:])
            gcnt_sb = cpool.tile([1, sch.ngath], mybir.dt.int32, tag="gcnt")
            nc.sync.dma_start(out=gcnt_sb[:], in_=gcnt_in[:])
            iotar_sb = cpool.tile([P, IOTAR_K * P], bf16, tag="iotar")
            nc.sync.dma_start(out=iotar_sb[:], in_=iotar_in[:])
            if not bias_zero:
                b1_sb = cpool.tile([1, hid], bf16, tag="b1r")
                nc.sync.dma_start(out=b1_sb[:], in_=b1r[:])
                b2_sb = cpool.tile([1, out_dim], bf16, tag="b2r")
                nc.sync.dma_start(out=b2_sb[:], in_=b2r[:])
                rdinv_sb = cpool.tile([1, nblk * P], bf16, tag="rdinvr")
                nc.sync.dma_start(out=rdinv_sb[:], in_=rdinvr[:])
            # SBUF-resident tables for self-loop adds
            hres = cpool.tile([P, nblk * hid], bf16, tag="hres")
            h2self = cpool.tile([P, nblk * out_dim], bf16, tag="h2self")

            # PSUM: matmul start=True zeroes the whole 2KB bank ("zero
            # region"), so each accumulation slot owns a full bank.
            psagg_t = [
                psagg.tile([P, 4 * P], f32, tag=f"agg{i}", name=f"aggbank{i}")
                for i in range(SB_N)
            ]

            def slot_ap(si, w):
                return psagg_t[si][:, :w]

            # ---- h~ = dinv * (x @ W1), shard-local, bf16
            XGRP = 8
            for g0 in range(0, nblk, XGRP):
                g1 = min(g0 + XGRP, nblk)
                c0, c1 = g0 * P, min(g1 * P, shard)
                xt = [
                    xpool.tile([P, XGRP * P], bf16, tag=f"xt{k}", name=f"xt{k}")
                    for k in range(kin)
                ]
                for k in range(kin):
                    nc.sync.dma_start(
                        out=xt[k][:, : c1 - c0], in_=xT[k * P : (k + 1) * P, c0:c1]
                    )
                for b in range(g0, g1):
                    m = min(P, shard - b * P)
                    hp = psagg_t[b % 4]
                    for k in range(kin):
                        nc.tensor.matmul(
                            out=hp[:m, :hid],
                            lhsT=xt[k][:, b * P - c0 : b * P - c0 + m],
                            rhs=w1_t[k][:],
                            start=(k == 0),
                            stop=(k == kin - 1),
                        )
                    nc.scalar.activation(
                        out=hres[:m, b * hid : b * hid + hid],
                        in_=hp[:m, :hid],
                        func=mybir.ActivationFunctionType.Copy,
                        scale=dinvb_sb[:m, b : b + 1],
                    )
                    nc.sync.dma_start(
                        out=hloc[b * P : b * P + m, :],
                        in_=hres[:m, b * hid : b * hid + hid],
                    )

            nc.gpsimd.collective_compute(
                "AllGather",
                mybir.AluOpType.bypass,
                ins=[hloc[:]],
                outs=[hfull[:]],
                replica_groups=[list(range(NCORES))],
            )

            gq = [0]
            h2s_init = [0]

            def run_layer(layer):
                table = hfull if layer == 1 else h2full
                w = P if layer == 1 else out_dim
                for s in range(n_sb):
                    sb = sch.sbs[s]
                    blocks = sb["blocks"]
                    slot_of = {b: i for i, b in enumerate(blocks)}
                    # self-loop add opens each block's accumulation
                    for b in blocks:
                        m = min(P, shard - b * P)
                        si = slot_of[b]
                        if layer == 1:
                            nc.tensor.matmul(
                                out=slot_ap(si, P),
                                lhsT=hres[:m, b * hid : b * hid + hid],
                                rhs=ident_sb[:m, :],
                                start=True,
                                stop=False,
                            )
                        else:
                            nc.tensor.matmul(
                                out=slot_ap(si, out_dim),
                                lhsT=ident_sb[:m, :],
                                rhs=h2self[:m, b * out_dim : (b + 1) * out_dim],
                                start=True,
                                stop=False,
                            )
                        if not bias_zero:
                            # rank-1 bias: b1 (x) 1/dinv  /  b2/dinv
                            if layer == 1:
                                nc.tensor.matmul(
                                    out=slot_ap(si, P),
                                    lhsT=b1_sb[:, :],
                                    rhs=rdinv_sb[:, b * P : (b + 1) * P],
                                    start=False,
                                    stop=False,
                                )
                            else:
                                nc.tensor.matmul(
                                    out=slot_ap(si, out_dim),
                                    lhsT=rdinv_sb[:, b * P : (b + 1) * P],
                                    rhs=b2_sb[:, :],
                                    start=False,
                                    stop=False,
                                )
                    for g in sb["gathers"]:
                        c16, nidx, k = g["c16"], g["nidx"], g["bank"]
                        gi = g["gi"]
                        nch = nidx // P
                        ninst_g = len(g["insts"])
                        it = ipool.tile(
                            [P, GMAX_CHUNKS * 8], mybir.dt.int16, tag="it"
                        )
                        nc.sync.dma_start(
                            out=it[:, : nidx // 16],
                            in_=idxs[:, c16 : c16 + nidx // 16],
                        )
                        gt = gpool.tile([P, GMAX_CHUNKS, P], bf16, tag="gt")
                        if gq[0] < 7:
                            # first use of each pool buffer: clear stale SBUF
                            # so trimmed (unwritten) slots stay finite
                            nc.vector.memset(gt[:], 0.0)
                        r0 = k * sch.bank_rows
                        r1 = min(r0 + sch.bank_rows, n)
                        nc.gpsimd.reg_load(nreg, gcnt_sb[:1, gi : gi + 1])
                        nc.gpsimd.dma_gather(
                            out_ap=gt[:, :nch, :],
                            in_ap=table[r0:r1, :],
                            idxs_ap=it[:, : nidx // 16],
                            num_idxs=nidx,
                            num_idxs_reg=nreg,
                            elem_size=P,
                            single_packet=False,
                            queue_num=gq[0] % NQUEUES,
                        )
                        gq[0] += 1
                        # batched indicator build for all instances
                        sbig = spool.tile([P, IOTAR_K, P], bf16, tag="sind")
                        nc.vector.tensor_tensor(
                            out=sbig[:, :ninst_g, :],
                            in0=iotar_sb[:, : ninst_g * P].rearrange(
                                "p (k f) -> p k f", k=ninst_g
                            ),
                            in1=dstloc_sb[
                                :, g["i0"] : g["i0"] + ninst_g
                            ].to_broadcast([P, ninst_g, P]),
                            op=mybir.AluOpType.is_equal,
                        )
                        for ii, (j, b) in enumerate(g["insts"]):
                            si = slot_of[b]
                            sp = g["stops"][ii]
                            if layer == 1:
                                nc.tensor.matmul(
                                    out=slot_ap(si, P),
                                    lhsT=gt[:, j, :],
                                    rhs=sbig[:, ii, :],
                                    start=False,
                                    stop=sp,
                                )
                            else:
                                nc.tensor.matmul(
                                    out=slot_ap(si, out_dim),
                                    lhsT=sbig[:, ii, :],
                                    rhs=gt[:, j, :out_dim],
                                    start=False,
                                    stop=sp,
                                )
                    # ---- block epilogues (Scalar + TensorE only)
                    # L1: all relus first — each relu is the PSUM read that
                    # frees its slot; keeping them ahead of the (matmul-gated)
                    # h2s/h2self copies in the scalar queue releases all
                    # slots for the next superblock immediately.
                    o1_t = {}
                    if layer == 1:
                        for b in blocks:
                            o1 = bpool.tile([P, P], bf16, tag="o1")
                            nc.scalar.activation(
                                out=o1[:],
                                in_=slot_ap(slot_of[b], P),
                                func=mybir.ActivationFunctionType.Relu,
                            )
                            o1_t[b] = o1
                    for b in blocks:
                        m = min(P, shard - b * P)
                        si = slot_of[b]
                        if layer == 1:
                            o1 = o1_t[b]
                            # W2 matmul reuses this block's own psagg bank
                            # (freed by the relu above) so the 7 epilogues
                            # don't serialize through one PSUM buffer.
                            h2p = slot_ap(si, out_dim)
                            nc.tensor.matmul(
                                out=h2p[:m, :],
                                lhsT=o1[:, :m],
                                rhs=w2_sb[:],
                                start=True,
                                stop=True,
                            )
                            h2s = bpool.tile([P, P], bf16, tag="h2s")
                            if h2s_init[0] < 16:
                                # pad cols are written once per pool buffer
                                # and stay zero (copies touch only [:, :64])
                                nc.vector.memset(h2s[:, out_dim:], 0.0)
                                h2s_init[0] += 1
                            nc.scalar.activation(
                                out=h2s[:m, :out_dim],
                                in_=h2p[:m, :],
                                func=mybir.ActivationFunctionType.Copy,
                                scale=dinvsq_sb[:m, b : b + 1],
                            )
                            nc.scalar.activation(
                                out=h2self[:m, b * out_dim : (b + 1) * out_dim],
                                in_=h2p[:m, :],
                                func=mybir.ActivationFunctionType.Copy,
                                scale=dinvsq_sb[:m, b : b + 1],
                            )
                            nc.sync.dma_start(
                                out=h2loc[b * P : b * P + m, :], in_=h2s[:m, :]
                            )
                        else:
                            ob = bpool.tile([P, out_dim], f32, tag="ob")
                            nc.scalar.activation(
                                out=ob[:m, :],
                                in_=slot_ap(si, out_dim)[:m, :],
                                func=mybir.ActivationFunctionType.Copy,
                                scale=dinvb_sb[:m, b : b + 1],
                            )
                            nc.sync.dma_start(
                                out=out_ext[b * P : b * P + m, :], in_=ob[:m, :]
                            )

            run_layer(1)
            nc.gpsimd.collective_compute(
                "AllGather",
                mybir.AluOpType.bypass,
                ins=[h2loc[:]],
                outs=[h2full[:]],
                replica_groups=[list(range(NCORES))],
            )
            run_layer(2)
            regstack.close()

    nc.compile()
    return nc


# ---------------------------------------------------------------- kernel ---
def _make_in_maps(sch, x, W1, b1v, W2, b2v, bias_zero):
    hid = W1.shape[1]
    out_dim = W2.shape[1]
    shard = sch.shard
    nblk = sch.nblk
    bf = ml_dtypes.bfloat16
    in_maps = []
    w1b = W1.astype(bf)
    w2b = W2.astype(bf)
    iotar = np.tile(np.arange(P, dtype=np.float32), (P, IOTAR_K)).astype(bf)
    ident = np.eye(P, dtype=np.float32).astype(bf)
    for c in range(NCORES):
        xs = np.ascontiguousarray(x[c * shard : (c + 1) * shard].astype(bf).T)
        dv = sch.dinv[c * shard : (c + 1) * shard].astype(np.float64)
        full = np.zeros(nblk * P, np.float64)
        full[:shard] = dv
        cols = np.ascontiguousarray(full.reshape(nblk, P).T)
        m = {
            "xT": xs,
            "idxs": sch.idx_stream[c],
            "dstloc": sch.dstloc_s[c],
            "gcnt": sch.gcnt[c].reshape(1, -1),
            "iotar": iotar,
            "ident": ident,
            "dinvb": cols.astype(np.float32),
            "dinvsq": (cols**2).astype(np.float32),
            "W1": w1b,
            "W2": w2b,
        }
        if not bias_zero:
            rd = np.zeros(nblk * P, np.float64)
            rd[:shard] = 1.0 / dv
            m["b1r"] = b1v.reshape(1, hid).astype(bf)
            m["b2r"] = b2v.reshape(1, out_dim).astype(bf)
            m["rdinvr"] = rd.reshape(1, nblk * P).astype(bf)
        in_maps.append(m)
    return in_maps


def _get_compiled(n, e, edge_index, in_dim, hid, out_dim, bias_zero):
    key = ("nc", n, e, bias_zero)
    if key not in _CACHE:
        sch = _preprocess(n, edge_index)
        _CACHE[("sched", n, e)] = sch
        _CACHE[key] = _build(sch, in_dim, hid, out_dim, bias_zero)
    return _CACHE[("sched", n, e)], _CACHE[key]


def kernel(x, edge_index, W1, b1, W2, b2):
    _install_compat()
    from concourse.bass_utils import run_bass_kernel_spmd

    x = np.asarray(x)
    edge_index = np.asarray(edge_index)
    W1 = np.asarray(W1, np.float32)
    b1v = np.asarray(b1, np.float32)
    W2 = np.asarray(W2, np.float32)
    b2v = np.asarray(b2, np.float32)
    n, in_dim = x.shape
    hid = W1.shape[1]
    out_dim = W2.shape[1]
    bias_zero = bool(np.all(b1v == 0) and np.all(b2v == 0))

    sch, nc = _get_compiled(
        n, edge_index.shape[1], edge_index, in_dim, hid, out_dim, bias_zero
    )
    in_maps = _make_in_maps(sch, x, W1, b1v, W2, b2v, bias_zero)
    import os

    trace = bool(os.environ.get("GCN_TRACE"))
    res = run_bass_kernel_spmd(
        nc, in_maps, core_ids=list(range(NCORES)), trace=trace
    )
    global LAST_EXEC_NS
    LAST_EXEC_NS = res.exec_time_ns
    return np.concatenate([res.results[c]["out"] for c in range(NCORES)], axis=0)


LAST_EXEC_NS = None
